# revision 33
# baseline (speedup 1.0000x reference)
"""Trainium2 Bass kernel for GQA attention (B=4, T=2048, D=2048, 16 heads / 4 kv groups, RoPE).

Sharding: 8 cores = 4 batches x 2 head-halves. Core c handles batch c//2 and
heads (c%2)*8..+8 with kv groups (c%2)*2..+2.

Structure (vs v1 baseline, 659us -> ~583us):
  - phase 1: per chunk [V-proj pairs -> K-proj -> Q-proj] channel-major with
    RoPE fused into psum eviction (DVE); V eviction on ACT; 1/sqrt(d) folded
    into the exp activation scale so only 2 unscaled fp16 rope tables load.
  - chunk 3 defers its Q-projection: the 8 Q m-groups are interleaved as PE
    filler into chunk-0's attention head slots (which are otherwise exp/ACT
    bound), after K3/V3 complete.
  - attention: per (qc,h): S^T tiles [k=128, q=512x2] -> exp(scale*s) on ACT
    -> PV via ones-augmented v (denominator in psum col 128), software
    pipelined one ktp ahead, with o_proj matmuls + PE transposes of the
    previous chunk interleaved as per-ktp filler so the PE never waits on ACT;
    each head's normalize emitted at its slot end to free the pv banks early.
  - psum: "mm" 2x[128,1024] scores, "op" 2x[128,512] proj/oproj/transpose-
    scratch, "pv" 1x[128,1024] packed pv regions = exactly 8 banks (a psum
    accumulation group's start=True clears its whole bank's has_written bits,
    so regions sharing a bank share one accumulation group).
  - DMA ordered for the per-core HBM limit: xc0+wvk first (vproj gate) on
    sync/gpsimd, then tables/wq, then later chunks; outputs alternate
    sync/gpsimd queues; the scalar (ACT) stream carries no DMAs.
All matmuls bf16 with fp32 PSUM accumulation; bf16 device output, host sums
the two half-core partials in fp32.
"""

import numpy as np
import ml_dtypes

BF16 = ml_dtypes.bfloat16

D_MODEL = 2048
NUM_HEADS = 16
QUERY_GROUPS = 4
HEAD_DIM = 128
B = 4
T = 2048
THETA = 10000.0
SCALE = 0.08838834764831845
N_CORES = 8

P = 128
NH = NUM_HEADS // 2          # 8 q heads per core
NG = QUERY_GROUPS // 2       # 2 kv groups per core
QDIM = NH * HEAD_DIM         # 1024
GDIM = NG * HEAD_DIM         # 256
NKT = D_MODEL // P           # 16 contraction tiles over d_model
NTT = T // P                 # 16 tiles over sequence
NCH = T // 512               # 4 chunks of 512 over sequence
NDT = QDIM // P              # 8 head/dim tiles per core

# f32 col offsets of the 4 pv regions in the pv psum tile. j0/j1 share bank A
# (cols 0-511), j2/j3 share bank B (cols 512-1023); each bank's two regions
# form ONE psum accumulation group (start only on the bank's first matmul)
# because start=True clears the whole bank's has_written bits.
OFFJ = [0, 132, 512, 644]
SCRW = 64                    # f32 cols per bf16 [128,128] transpose scratch
VSTR = 2 * (P + 1)           # 258 bf16 cols per t-tile block in vbig


def build_nc(masked: bool):
    import concourse.bacc as bacc
    import concourse.tile as tile
    import concourse.mybir as mybir
    from concourse.masks import make_identity
    from contextlib import ExitStack

    dt = mybir.dt
    f32 = dt.float32
    bf16 = dt.bfloat16
    AF = mybir.ActivationFunctionType

    nc = bacc.Bacc("TRN2", target_bir_lowering=False, debug=False, num_devices=N_CORES)

    xt = nc.dram_tensor("xt", [D_MODEL, T], bf16, kind="ExternalInput")
    wq = nc.dram_tensor("wq", [D_MODEL, QDIM], bf16, kind="ExternalInput")
    # wv|wk packed so the tiles have 1KB lines (512B-line DMAs run ~2x slower)
    wvk = nc.dram_tensor("wvk", [D_MODEL, 2 * GDIM], bf16, kind="ExternalInput")
    wo = nc.dram_tensor("wo", [QDIM, D_MODEL], bf16, kind="ExternalInput")
    cos2 = nc.dram_tensor("cos2", [P, T], dt.float16, kind="ExternalInput")
    sinA = nc.dram_tensor("sinA", [P, T], dt.float16, kind="ExternalInput")
    if masked:
        maskcol = nc.dram_tensor("maskcol", [P, NTT], f32, kind="ExternalInput")
    out = nc.dram_tensor("out", [T, D_MODEL], bf16, kind="ExternalOutput")

    with tile.TileContext(nc) as tc:
        with ExitStack() as ctx:
            psum = ctx.enter_context(tc.tile_pool(name="ps", bufs=1, space="PSUM"))
            constp = ctx.enter_context(tc.tile_pool(name="const", bufs=1))
            qkT_pool = ctx.enter_context(tc.tile_pool(name="qkT", bufs=NH + NG))
            vbig_pool = ctx.enter_context(tc.tile_pool(name="vbig", bufs=1))
            attn_pool = ctx.enter_context(tc.tile_pool(name="attn", bufs=NTT))
            pt_pool = ctx.enter_context(tc.tile_pool(name="pt", bufs=4))
            rc_pool = ctx.enter_context(tc.tile_pool(name="rc", bufs=8))

            def mm_tile():
                return psum.tile([P, 1024], f32, tag="mm", bufs=2, name="mmt")

            def op_tile():
                return psum.tile([P, 512], f32, tag="op", bufs=2, name="opt")

            def pv_tile():
                return psum.tile([P, 1024], f32, tag="pv", bufs=1, name="pvt")

            identity = constp.tile([P, P], bf16, tag="identity")
            make_identity(nc, identity[:])
            dummy = constp.tile([P, 256], bf16, tag="dummy")
            nc.vector.memset(dummy[:], 0.0)
            if masked:
                maskcol_t = constp.tile([P, NTT], f32, tag="maskcol")
                nc.gpsimd.dma_start(out=maskcol_t[:], in_=maskcol[:, :])

            # persistent bf16 tensors
            # qkT[0..7] = q heads, qkT[8..9] = k groups; [d=128, T] channel-major
            qkT = [qkT_pool.tile([P, T], bf16, tag="qkT", name=f"qkT{i}")
                   for i in range(NH + NG)]
            # vbig: per t-tile block of 258 cols: [v_g0(128) | ones | v_g1(128) | ones]
            vbig = vbig_pool.tile([P, NTT * VSTR], bf16, tag="vbig")
            for i in range(NTT):
                for g in range(NG):
                    c = i * VSTR + g * (P + 1) + P
                    nc.vector.memset(vbig[:, c:c + 1], 1.0)
            attn_t = [attn_pool.tile([P, QDIM], bf16, tag="attn", name=f"attn{i}")
                      for i in range(NTT)]

            # ---------------- warmup (HAM + cover initial DMA latency) ------
            warm = op_tile()
            for i in range(48):
                nc.tensor.matmul(warm[:, 0:256], lhsT=identity[:], rhs=dummy[:],
                                 start=(i == 0), stop=(i == 47))
            wsink = constp.tile([P, 16], f32, tag="wsink")
            nc.vector.tensor_copy(wsink[:], warm[:, 0:16])

            # ---------------- shared attention machinery -------------------
            def normalize(qc, h, pvs_prev):
                # attn_t[qt][:, h*128:+128] = pvs[:, j] / denom  (DVE)
                for j in range(4):
                    qt = qc * 4 + j
                    rc = rc_pool.tile([P, 1], f32, tag="rc", name="rc")
                    nc.vector.reciprocal(
                        rc[:], pvs_prev[:, OFFJ[j] + P:OFFJ[j] + P + 1])
                    nc.vector.tensor_scalar_mul(
                        attn_t[qt][:, h * P:(h + 1) * P],
                        pvs_prev[:, OFFJ[j]:OFFJ[j] + P], rc[:])

            def attn_slot(qc, h, fillers):
                """One head slot: scores+exp+PV pipelined 1 ktp deep, with
                `fillers` (list of <=8 callables of ~0.5-1us PE work each)
                emitted at the 8 interleave points."""
                g = h // 4
                kT = qkT[NH + g]
                qT = qkT[h]
                c0 = qc * 512

                pvs = pv_tile()

                ps_l = [None] * 8
                pt_l = [None] * 8

                def emit_mms(ktp):
                    ps = mm_tile()
                    ps_l[ktp] = ps
                    for s in range(2):
                        kt = ktp * 2 + s
                        nc.tensor.matmul(
                            ps[:, s * 512:(s + 1) * 512],
                            lhsT=kT[:, kt * P:(kt + 1) * P],
                            rhs=qT[:, c0:c0 + 512],
                            start=True, stop=True)
                    pt = pt_pool.tile([P, 1024], bf16, tag="pt", name="pt")
                    pt_l[ktp] = pt
                    if masked:
                        for s in range(2):
                            kt = ktp * 2 + s
                            nc.scalar.activation(
                                pt[:, s * 512:(s + 1) * 512],
                                ps[:, s * 512:(s + 1) * 512],
                                AF.Exp, bias=maskcol_t[:, kt:kt + 1], scale=SCALE)
                    else:
                        nc.scalar.activation(pt[:], ps[:], AF.Exp, scale=SCALE)

                def emit_pv(ktp):
                    pt = pt_l[ktp]
                    for s in range(2):
                        kt = ktp * 2 + s
                        for j in range(4):
                            # j0/j1 (bank A) and j2/j3 (bank B) each form one
                            # accumulation group: start only on the bank's
                            # first matmul, stop on its last.
                            nc.tensor.matmul(
                                pvs[:, OFFJ[j]:OFFJ[j] + P + 1],
                                lhsT=pt[:, s * 512 + j * P:s * 512 + (j + 1) * P],
                                rhs=vbig[:, kt * VSTR + g * (P + 1):
                                         kt * VSTR + (g + 1) * (P + 1)],
                                start=(kt == 0 and j % 2 == 0),
                                stop=(kt == NTT - 1 and j % 2 == 1),
                                skip_group_check=True)

                emit_mms(0)
                for ktp in range(8):
                    if ktp < 7:
                        emit_mms(ktp + 1)
                    if ktp < len(fillers) and fillers[ktp] is not None:
                        fillers[ktp]()
                    emit_pv(ktp)

                # emit this head's normalize now so it sits ahead of the next
                # slot's filler casts in the DVE queue: the pv banks then free
                # ~1us into the next slot, before its first PV matmul.
                normalize(qc, h, pvs)

            # ---------------- phase 1: projections + rope -------------------
            with ExitStack() as ph1:
                wq_pool = ph1.enter_context(tc.tile_pool(name="wq", bufs=NKT))
                wvk_pool = ph1.enter_context(tc.tile_pool(name="wvk", bufs=NKT))
                xc_pool = ph1.enter_context(tc.tile_pool(name="xc", bufs=32))
                tab_pool = ph1.enter_context(tc.tile_pool(name="tab", bufs=1))
                tmp_pool = ph1.enter_context(tc.tile_pool(name="rtmp", bufs=2))

                # --- DMA emission: sync + gpsimd queues only, so the scalar
                # (ACT) instruction stream stays free for psum evictions ---
                xc_tiles = {}

                def load_xc(nch, kts, eng):
                    c0 = nch * 512
                    for kt in kts:
                        tl = xc_pool.tile([P, 512], bf16, tag="xc", name=f"xc{nch}_{kt}")
                        eng.dma_start(out=tl[:], in_=xt[kt * P:(kt + 1) * P, c0:c0 + 512])
                        xc_tiles[(nch, kt)] = tl

                # per-core HBM (~358 GB/s) is the startup wall: put ONLY the
                # vproj gate (xc0 on sync, wvk on gpsimd) in the first window,
                # then tables, then wq, then xc1.
                load_xc(0, range(NKT), nc.sync)
                wvk_t = []
                for kt in range(NKT):
                    tl = wvk_pool.tile([P, 2 * GDIM], bf16, tag="wvk", name=f"wvkt{kt}")
                    nc.gpsimd.dma_start(out=tl[:], in_=wvk[kt * P:(kt + 1) * P, :])
                    wvk_t.append(tl)
                wv_t = [tl[:, 0:GDIM] for tl in wvk_t]
                wk_t = [tl[:, GDIM:2 * GDIM] for tl in wvk_t]
                cos_t = tab_pool.tile([P, T], dt.float16, tag="cos", name="cos_t")
                nc.gpsimd.dma_start(out=cos_t[:], in_=cos2[:, :])
                sin_t = tab_pool.tile([P, T], dt.float16, tag="sin", name="sin_t")
                nc.gpsimd.dma_start(out=sin_t[:], in_=sinA[:, :])
                wq_t = [wq_pool.tile([P, QDIM], bf16, tag="wq", name=f"wqt{kt}")
                        for kt in range(NKT)]
                for kt in range(6):
                    nc.sync.dma_start(out=wq_t[kt][:], in_=wq[kt * P:(kt + 1) * P, :])
                for kt in range(6, NKT):
                    nc.gpsimd.dma_start(out=wq_t[kt][:], in_=wq[kt * P:(kt + 1) * P, :])
                load_xc(1, range(NKT), nc.sync)

                def vproj_pair(c, pair):
                    # two t-tiles of 128 into one [128,512] psum; ACT evicts
                    ps = op_tile()
                    for tl_i in (0, 1):
                        for kt in range(NKT):
                            nc.tensor.matmul(
                                ps[:, tl_i * 256:tl_i * 256 + GDIM],
                                lhsT=xc_tiles[(c, kt)][:, (pair * 2 + tl_i) * P:
                                                       (pair * 2 + tl_i + 1) * P],
                                rhs=wv_t[kt],
                                start=(kt == 0), stop=(kt == NKT - 1))
                    for tl_i in (0, 1):
                        tglob = c * 4 + pair * 2 + tl_i
                        for g in range(NG):
                            nc.scalar.activation(
                                vbig[:, tglob * VSTR + g * (P + 1):
                                     tglob * VSTR + g * (P + 1) + P],
                                ps[:, tl_i * 256 + g * P:tl_i * 256 + (g + 1) * P],
                                AF.Copy)

                def rope_evict(m, ps, c0):
                    # qkT[m][:, c0:c0+512] = ps*cos2 + rot(ps)*sinA   (DVE, f32)
                    t1 = tmp_pool.tile([P, 512], f32, tag="t1", name="t1")
                    t2 = tmp_pool.tile([P, 512], f32, tag="t2", name="t2")
                    h2 = P // 2
                    nc.vector.tensor_mul(t1[:], ps[:], cos_t[:, c0:c0 + 512])
                    nc.vector.tensor_mul(
                        t2[0:h2, :], ps[h2:P, :], sin_t[0:h2, c0:c0 + 512])
                    nc.vector.tensor_mul(
                        t2[h2:P, :], ps[0:h2, :], sin_t[h2:P, c0:c0 + 512])
                    nc.vector.tensor_add(qkT[m][:, c0:c0 + 512], t1[:], t2[:])

                def kq_mgroup(c, m):
                    # m 0..7 -> q head m (wq cols), m 8..9 -> k group (wk cols)
                    ps = op_tile()
                    for kt in range(NKT):
                        if m < NH:
                            lhsT = wq_t[kt][:, m * P:(m + 1) * P]
                        else:
                            lhsT = wk_t[kt][:, (m - NH) * P:(m - NH + 1) * P]
                        nc.tensor.matmul(
                            ps[:], lhsT=lhsT,
                            rhs=xc_tiles[(c, kt)][:],
                            start=(kt == 0), stop=(kt == NKT - 1))
                    rope_evict(m, ps, c * 512)

                for c in range(NCH):
                    if 1 <= c < NCH - 1:
                        load_xc(c + 1, range(NKT), nc.sync)
                    vproj_pair(c, 0)
                    vproj_pair(c, 1)
                    # K first (enables attention right after chunk 3's K)
                    for m in (NH, NH + 1):
                        kq_mgroup(c, m)
                    if c < NCH - 1:
                        for m in range(NH):
                            kq_mgroup(c, m)

                # ---- hybrid: chunk-0 attention, Q3 m-groups as PE filler ----
                q3 = {}

                def q3_unit(h, quarter):
                    # quarter of Q-projection m-group h for chunk 3 (4 MMs)
                    c = NCH - 1
                    if quarter == 0:
                        q3[h] = op_tile()
                    ps = q3[h]
                    for kt in range(quarter * 4, quarter * 4 + 4):
                        nc.tensor.matmul(
                            ps[:], lhsT=wq_t[kt][:, h * P:(h + 1) * P],
                            rhs=xc_tiles[(c, kt)][:],
                            start=(kt == 0), stop=(kt == NKT - 1))
                    if quarter == 3:
                        rope_evict(h, ps, c * 512)

                for h in range(NH):
                    fillers = [None] * 8
                    for q in range(4):
                        fillers[1 + 2 * q] = (lambda hh=h, qq=q: q3_unit(hh, qq))
                    attn_slot(0, h, fillers)

            # ---------------- phase 2: chunks 1-3 + o_proj ------------------
            wo_pool = ctx.enter_context(tc.tile_pool(name="wo", bufs=NDT))
            aT_pool = ctx.enter_context(tc.tile_pool(name="aT", bufs=1))
            osb_pool = ctx.enter_context(tc.tile_pool(name="osb", bufs=6))

            # aTbig[:, dtile*T + qt*128 : +128] = attn_t[qt][:, dtile].T
            aTbig = aT_pool.tile([P, NDT * T], bf16, tag="aT")
            wo_t = []
            for dtile in range(NDT):
                tl = wo_pool.tile([P, D_MODEL], bf16, tag="wo", name=f"wot{dtile}")
                wo_t.append(tl)
            for dtile in range(NDT):
                nc.gpsimd.dma_start(out=wo_t[dtile][:, 0:1024],
                                    in_=wo[dtile * P:(dtile + 1) * P, 0:1024])
            for dtile in range(NDT):
                nc.gpsimd.dma_start(out=wo_t[dtile][:, 1024:2048],
                                    in_=wo[dtile * P:(dtile + 1) * P, 1024:2048])

            scr_state = {"tile": None}

            def transp_unit(qcp, tt, pair):
                # transpose attn_t[qt] dtiles (2*pair, 2*pair+1) into aTbig.
                # One native-bf16 psum tile in the op tag (same 2KB slot size)
                # holds all 8 transposes of a t-tile: plain slices keep the
                # subtile dep tracking precise (a bitcast view here serialized
                # every transpose against the previous region's DVE copy).
                # The transpose's start=True bank-clear cannot disturb any
                # in-flight accumulation since the tile owns its bank.
                qt = qcp * 4 + tt
                if pair == 0:
                    scr_state["tile"] = psum.tile(
                        [P, NDT * P], bf16, tag="op", bufs=2, name="scrt")
                scrt = scr_state["tile"]
                for s2 in range(2):
                    dtile = pair * 2 + s2
                    scr = scrt[:, dtile * P:(dtile + 1) * P]
                    nc.tensor.transpose(
                        scr, attn_t[qt][:, dtile * P:(dtile + 1) * P], identity[:])
                    nc.vector.tensor_copy(
                        aTbig[:, dtile * T + qt * P:dtile * T + (qt + 1) * P], scr)

            op_state = {}

            def oproj_unit(tt, nchn, half):
                # half 0: dtiles 0-3 (start); half 1: dtiles 4-7 (stop+evict)
                if half == 0:
                    op_state[(tt, nchn)] = op_tile()
                ps = op_state[(tt, nchn)]
                for dtile in range(half * 4, half * 4 + 4):
                    nc.tensor.matmul(
                        ps[:],
                        lhsT=aTbig[:, dtile * T + tt * P:dtile * T + (tt + 1) * P],
                        rhs=wo_t[dtile][:, nchn * 512:(nchn + 1) * 512],
                        start=(dtile == 0), stop=(dtile == NDT - 1))
                if half == 1:
                    del op_state[(tt, nchn)]
                    osb = osb_pool.tile([P, 512], bf16, tag="osb", name="osb")
                    # evict on ACT: frees the op bank without queueing behind
                    # the slot's DVE work (normalize + transpose copies)
                    nc.scalar.activation(osb[:], ps[:], AF.Copy)
                    # alternate output queues (sync is idle during phase 2)
                    eng = nc.gpsimd if (tt + nchn) % 2 == 0 else nc.sync
                    eng.dma_start(
                        out=out[tt * P:(tt + 1) * P, nchn * 512:(nchn + 1) * 512],
                        in_=osb[:])

            # filler scheduling: per chunk qc (1..3), slots h=0..7 carry
            # transposes of chunk qc-1 (slot h<4 -> tt=h, 4 pair-units at
            # points 4-7) and o_proj groups of chunk qc-1 (2 units each) from
            # a readiness queue.
            ready_groups = []   # (tt_glob, nchn) ready once tt transposed

            for qc in range(1, NCH):
                qcp = qc - 1
                for h in range(NH):
                    fillers = []
                    trans = []
                    if h < 4:
                        trans = [(lambda t=h, p=p2: transp_unit(qcp, t, p))
                                 for p2 in range(4)]
                    # in the last chunk, hold back one group in slots 2-3
                    # (mid-chunk, where the underfill is too small to trip the
                    # HAM throttle) so ~4 dependency-free groups remain for
                    # the tail to chew on while the attention pipeline drains
                    n_op = min(8 - len(trans), 4)
                    if qc == NCH - 1 and h in (2, 3):
                        n_op = 2
                    opu = []
                    while ready_groups and len(opu) + 2 <= n_op:
                        ttg, nchn = ready_groups.pop(0)
                        opu.append(lambda a=ttg, b=nchn: oproj_unit(a, b, 0))
                        opu.append(lambda a=ttg, b=nchn: oproj_unit(a, b, 1))
                    # op units first (points 0..), transposes at the tail
                    fillers = opu + trans
                    attn_slot(qc, h, fillers)
                    if h < 4:
                        ttg = qcp * 4 + h
                        for nchn in range(NCH):
                            ready_groups.append((ttg, nchn))

            # ---------------- tail: transposes + o_proj of the last chunk ---
            # first flush leftover (long-ready) groups so the PE has work
            # while the last head's exp/PV/normalize chain drains
            leftovers, ready_groups = ready_groups, []
            for ttg, nchn in leftovers:
                oproj_unit(ttg, nchn, 0)
                oproj_unit(ttg, nchn, 1)
            qcp = NCH - 1
            for tt in range(4):
                for p2 in range(4):
                    transp_unit(qcp, tt, p2)
                for nchn in range(NCH):
                    ready_groups.append((qcp * 4 + tt, nchn))
            for ttg, nchn in ready_groups:
                oproj_unit(ttg, nchn, 0)
                oproj_unit(ttg, nchn, 1)

    nc.compile()
    return nc


def make_tables():
    inv_freq = 1.0 / (THETA ** (np.arange(0, HEAD_DIM, 2, dtype=np.float32)
                                / HEAD_DIM))          # [64]
    ang = np.arange(T, dtype=np.float32)[:, None] * inv_freq[None, :]  # [T, 64]
    cos = np.cos(ang).T.astype(np.float32)            # [64, T]
    sin = np.sin(ang).T.astype(np.float32)
    cos2 = np.concatenate([cos, cos], axis=0)         # [128, T]
    sinA = np.concatenate([-sin, sin], axis=0)        # [128, T]
    return (np.ascontiguousarray(cos2).astype(np.float16),
            np.ascontiguousarray(sinA).astype(np.float16))


def make_in_maps(x, W_qkv, W_o, padding_mask, masked):
    cos2_v, sinA_v = make_tables()
    in_maps = []
    for c in range(N_CORES):
        b, half = c // 2, c % 2
        q0 = half * QDIM
        k0 = NUM_HEADS * HEAD_DIM + half * GDIM
        v0 = NUM_HEADS * HEAD_DIM + QUERY_GROUPS * HEAD_DIM + half * GDIM
        wvk_v = np.concatenate(
            [W_qkv[:, v0:v0 + GDIM], W_qkv[:, k0:k0 + GDIM]], axis=1)
        m = {
            "xt": np.ascontiguousarray(x[b].T).astype(BF16),
            "wq": np.ascontiguousarray(W_qkv[:, q0:q0 + QDIM]).astype(BF16),
            "wvk": np.ascontiguousarray(wvk_v).astype(BF16),
            "wo": np.ascontiguousarray(W_o[half * QDIM:(half + 1) * QDIM, :]).astype(BF16),
            "cos2": cos2_v, "sinA": sinA_v,
        }
        if masked:
            bias = np.where(padding_mask[b], 0.0, -1e30).astype(np.float32)  # [T]
            m["maskcol"] = np.ascontiguousarray(
                bias.reshape(NTT, P).T).astype(np.float32)
        in_maps.append(m)
    return in_maps


_nc_cache = {}


def kernel(x, W_qkv, W_o, padding_mask, trace=False):
    from concourse.bass_utils import run_bass_kernel_spmd

    x = np.asarray(x)
    W_qkv = np.asarray(W_qkv)
    W_o = np.asarray(W_o)
    padding_mask = np.asarray(padding_mask)
    masked = not bool(padding_mask.all())

    if masked not in _nc_cache:
        _nc_cache[masked] = build_nc(masked)
    nc = _nc_cache[masked]

    in_maps = make_in_maps(x, W_qkv, W_o, padding_mask, masked)
    res = run_bass_kernel_spmd(
        nc, in_maps, core_ids=list(range(N_CORES)),
        trace=trace, trace_cores=[0] if trace else None)

    out = np.empty((B, T, D_MODEL), np.float32)
    for b in range(B):
        out[b] = (res.results[2 * b]["out"].astype(np.float32)
                  + res.results[2 * b + 1]["out"].astype(np.float32))
    kernel.last_exec_time_ns = res.exec_time_ns
    kernel.last_results = res
    return out


# revision 34
# speedup vs baseline: 1.0024x; 1.0024x over previous
"""Trainium2 Bass kernel for GQA attention (B=4, T=2048, D=2048, 16 heads / 4 kv groups, RoPE).

Sharding: 8 cores = 4 batches x 2 head-halves. Core c handles batch c//2 and
heads (c%2)*8..+8 with kv groups (c%2)*2..+2.

Structure (vs v1 baseline, 659us -> ~583us):
  - phase 1: per chunk [V-proj pairs -> K-proj -> Q-proj] channel-major with
    RoPE fused into psum eviction (DVE); V eviction on ACT; 1/sqrt(d) folded
    into the exp activation scale so only 2 unscaled fp16 rope tables load.
  - chunk 3 defers its Q-projection: the 8 Q m-groups are interleaved as PE
    filler into chunk-0's attention head slots (which are otherwise exp/ACT
    bound), after K3/V3 complete.
  - attention: per (qc,h): S^T tiles [k=128, q=512x2] -> exp(scale*s) on ACT
    -> PV via ones-augmented v (denominator in psum col 128), software
    pipelined one ktp ahead, with o_proj matmuls + PE transposes of the
    previous chunk interleaved as per-ktp filler so the PE never waits on ACT;
    each head's normalize emitted at its slot end to free the pv banks early.
  - psum: "mm" 2x[128,1024] scores, "op" 2x[128,512] proj/oproj/transpose-
    scratch, "pv" 1x[128,1024] packed pv regions = exactly 8 banks (a psum
    accumulation group's start=True clears its whole bank's has_written bits,
    so regions sharing a bank share one accumulation group).
  - DMA ordered for the per-core HBM limit: xc0+wvk first (vproj gate) on
    sync/gpsimd, then tables/wq, then later chunks; outputs alternate
    sync/gpsimd queues; the scalar (ACT) stream carries no DMAs.
All matmuls bf16 with fp32 PSUM accumulation; bf16 device output, host sums
the two half-core partials in fp32.
"""

import numpy as np
import ml_dtypes

BF16 = ml_dtypes.bfloat16

D_MODEL = 2048
NUM_HEADS = 16
QUERY_GROUPS = 4
HEAD_DIM = 128
B = 4
T = 2048
THETA = 10000.0
SCALE = 0.08838834764831845
N_CORES = 8

P = 128
NH = NUM_HEADS // 2          # 8 q heads per core
NG = QUERY_GROUPS // 2       # 2 kv groups per core
QDIM = NH * HEAD_DIM         # 1024
GDIM = NG * HEAD_DIM         # 256
NKT = D_MODEL // P           # 16 contraction tiles over d_model
NTT = T // P                 # 16 tiles over sequence
NCH = T // 512               # 4 chunks of 512 over sequence
NDT = QDIM // P              # 8 head/dim tiles per core

# f32 col offsets of the 4 pv regions in the pv psum tile. j0/j1 share bank A
# (cols 0-511), j2/j3 share bank B (cols 512-1023); each bank's two regions
# form ONE psum accumulation group (start only on the bank's first matmul)
# because start=True clears the whole bank's has_written bits.
OFFJ = [0, 132, 512, 644]
SCRW = 64                    # f32 cols per bf16 [128,128] transpose scratch
VSTR = 2 * (P + 1)           # 258 bf16 cols per t-tile block in vbig


def build_nc(masked: bool):
    import concourse.bacc as bacc
    import concourse.tile as tile
    import concourse.mybir as mybir
    from concourse.masks import make_identity
    from contextlib import ExitStack

    dt = mybir.dt
    f32 = dt.float32
    bf16 = dt.bfloat16
    AF = mybir.ActivationFunctionType

    nc = bacc.Bacc("TRN2", target_bir_lowering=False, debug=False, num_devices=N_CORES)

    xt = nc.dram_tensor("xt", [D_MODEL, T], bf16, kind="ExternalInput")
    wq = nc.dram_tensor("wq", [D_MODEL, QDIM], bf16, kind="ExternalInput")
    # wv|wk packed so the tiles have 1KB lines (512B-line DMAs run ~2x slower)
    wvk = nc.dram_tensor("wvk", [D_MODEL, 2 * GDIM], bf16, kind="ExternalInput")
    wo = nc.dram_tensor("wo", [QDIM, D_MODEL], bf16, kind="ExternalInput")
    cos2 = nc.dram_tensor("cos2", [P, T], dt.float16, kind="ExternalInput")
    sinA = nc.dram_tensor("sinA", [P, T], dt.float16, kind="ExternalInput")
    if masked:
        maskcol = nc.dram_tensor("maskcol", [P, NTT], f32, kind="ExternalInput")
    out = nc.dram_tensor("out", [T, D_MODEL], bf16, kind="ExternalOutput")

    with tile.TileContext(nc) as tc:
        with ExitStack() as ctx:
            psum = ctx.enter_context(tc.tile_pool(name="ps", bufs=1, space="PSUM"))
            constp = ctx.enter_context(tc.tile_pool(name="const", bufs=1))
            qkT_pool = ctx.enter_context(tc.tile_pool(name="qkT", bufs=NH + NG))
            vbig_pool = ctx.enter_context(tc.tile_pool(name="vbig", bufs=1))
            attn_pool = ctx.enter_context(tc.tile_pool(name="attn", bufs=NTT))
            pt_pool = ctx.enter_context(tc.tile_pool(name="pt", bufs=4))
            rc_pool = ctx.enter_context(tc.tile_pool(name="rc", bufs=8))

            def mm_tile():
                return psum.tile([P, 1024], f32, tag="mm", bufs=2, name="mmt")

            def op_tile():
                return psum.tile([P, 512], f32, tag="op", bufs=2, name="opt")

            def pv_tile():
                return psum.tile([P, 1024], f32, tag="pv", bufs=1, name="pvt")

            identity = constp.tile([P, P], bf16, tag="identity")
            make_identity(nc, identity[:])
            dummy = constp.tile([P, 256], bf16, tag="dummy")
            nc.vector.memset(dummy[:], 0.0)
            if masked:
                maskcol_t = constp.tile([P, NTT], f32, tag="maskcol")
                nc.gpsimd.dma_start(out=maskcol_t[:], in_=maskcol[:, :])

            # persistent bf16 tensors
            # qkT[0..7] = q heads, qkT[8..9] = k groups; [d=128, T] channel-major
            qkT = [qkT_pool.tile([P, T], bf16, tag="qkT", name=f"qkT{i}")
                   for i in range(NH + NG)]
            # vbig: per t-tile block of 258 cols: [v_g0(128) | ones | v_g1(128) | ones]
            vbig = vbig_pool.tile([P, NTT * VSTR], bf16, tag="vbig")
            for i in range(NTT):
                for g in range(NG):
                    c = i * VSTR + g * (P + 1) + P
                    nc.vector.memset(vbig[:, c:c + 1], 1.0)
            attn_t = [attn_pool.tile([P, QDIM], bf16, tag="attn", name=f"attn{i}")
                      for i in range(NTT)]

            # ---------------- warmup (HAM + cover initial DMA latency) ------
            warm = op_tile()
            for i in range(48):
                nc.tensor.matmul(warm[:, 0:256], lhsT=identity[:], rhs=dummy[:],
                                 start=(i == 0), stop=(i == 47))
            wsink = constp.tile([P, 16], f32, tag="wsink")
            nc.vector.tensor_copy(wsink[:], warm[:, 0:16])

            # ---------------- shared attention machinery -------------------
            def normalize(qc, h, pvs_prev):
                # attn_t[qt][:, h*128:+128] = pvs[:, j] / denom  (DVE)
                for j in range(4):
                    qt = qc * 4 + j
                    rc = rc_pool.tile([P, 1], f32, tag="rc", name="rc")
                    nc.vector.reciprocal(
                        rc[:], pvs_prev[:, OFFJ[j] + P:OFFJ[j] + P + 1])
                    nc.vector.tensor_scalar_mul(
                        attn_t[qt][:, h * P:(h + 1) * P],
                        pvs_prev[:, OFFJ[j]:OFFJ[j] + P], rc[:])

            def attn_slot(qc, h, fillers):
                """One head slot: scores+exp+PV pipelined 1 ktp deep, with
                `fillers` (list of <=8 callables of ~0.5-1us PE work each)
                emitted at the 8 interleave points."""
                g = h // 4
                kT = qkT[NH + g]
                qT = qkT[h]
                c0 = qc * 512

                pvs = pv_tile()

                ps_l = [None] * 8
                pt_l = [None] * 8

                def emit_mms(ktp):
                    ps = mm_tile()
                    ps_l[ktp] = ps
                    for s in range(2):
                        kt = ktp * 2 + s
                        nc.tensor.matmul(
                            ps[:, s * 512:(s + 1) * 512],
                            lhsT=kT[:, kt * P:(kt + 1) * P],
                            rhs=qT[:, c0:c0 + 512],
                            start=True, stop=True)
                    pt = pt_pool.tile([P, 1024], bf16, tag="pt", name="pt")
                    pt_l[ktp] = pt
                    if masked:
                        for s in range(2):
                            kt = ktp * 2 + s
                            nc.scalar.activation(
                                pt[:, s * 512:(s + 1) * 512],
                                ps[:, s * 512:(s + 1) * 512],
                                AF.Exp, bias=maskcol_t[:, kt:kt + 1], scale=SCALE)
                    else:
                        nc.scalar.activation(pt[:], ps[:], AF.Exp, scale=SCALE)

                def emit_pv(ktp):
                    pt = pt_l[ktp]
                    for s in range(2):
                        kt = ktp * 2 + s
                        for j in range(4):
                            # j0/j1 (bank A) and j2/j3 (bank B) each form one
                            # accumulation group: start only on the bank's
                            # first matmul, stop on its last.
                            nc.tensor.matmul(
                                pvs[:, OFFJ[j]:OFFJ[j] + P + 1],
                                lhsT=pt[:, s * 512 + j * P:s * 512 + (j + 1) * P],
                                rhs=vbig[:, kt * VSTR + g * (P + 1):
                                         kt * VSTR + (g + 1) * (P + 1)],
                                start=(kt == 0 and j % 2 == 0),
                                stop=(kt == NTT - 1 and j % 2 == 1),
                                skip_group_check=True)

                emit_mms(0)
                for ktp in range(8):
                    if ktp < 7:
                        emit_mms(ktp + 1)
                    if ktp < len(fillers) and fillers[ktp] is not None:
                        fillers[ktp]()
                    emit_pv(ktp)

                # emit this head's normalize now so it sits ahead of the next
                # slot's filler casts in the DVE queue: the pv banks then free
                # ~1us into the next slot, before its first PV matmul.
                normalize(qc, h, pvs)

            # ---------------- phase 1: projections + rope -------------------
            with ExitStack() as ph1:
                wq_pool = ph1.enter_context(tc.tile_pool(name="wq", bufs=NKT))
                wvk_pool = ph1.enter_context(tc.tile_pool(name="wvk", bufs=NKT))
                xc_pool = ph1.enter_context(tc.tile_pool(name="xc", bufs=32))
                tab_pool = ph1.enter_context(tc.tile_pool(name="tab", bufs=1))
                tmp_pool = ph1.enter_context(tc.tile_pool(name="rtmp", bufs=2))

                # --- DMA emission: sync + gpsimd queues only, so the scalar
                # (ACT) instruction stream stays free for psum evictions ---
                xc_tiles = {}

                def load_xc(nch, kts, eng):
                    c0 = nch * 512
                    for kt in kts:
                        tl = xc_pool.tile([P, 512], bf16, tag="xc", name=f"xc{nch}_{kt}")
                        eng.dma_start(out=tl[:], in_=xt[kt * P:(kt + 1) * P, c0:c0 + 512])
                        xc_tiles[(nch, kt)] = tl

                # per-core HBM (~358 GB/s) is the startup wall: put ONLY the
                # vproj gate (xc0 on sync, wvk on gpsimd) in the first window,
                # then tables, then wq, then xc1.
                load_xc(0, range(NKT), nc.sync)
                wvk_t = []
                for kt in range(NKT):
                    tl = wvk_pool.tile([P, 2 * GDIM], bf16, tag="wvk", name=f"wvkt{kt}")
                    nc.gpsimd.dma_start(out=tl[:], in_=wvk[kt * P:(kt + 1) * P, :])
                    wvk_t.append(tl)
                wv_t = [tl[:, 0:GDIM] for tl in wvk_t]
                wk_t = [tl[:, GDIM:2 * GDIM] for tl in wvk_t]
                cos_t = tab_pool.tile([P, T], dt.float16, tag="cos", name="cos_t")
                nc.gpsimd.dma_start(out=cos_t[:], in_=cos2[:, :])
                sin_t = tab_pool.tile([P, T], dt.float16, tag="sin", name="sin_t")
                nc.gpsimd.dma_start(out=sin_t[:], in_=sinA[:, :])
                wq_t = [wq_pool.tile([P, QDIM], bf16, tag="wq", name=f"wqt{kt}")
                        for kt in range(NKT)]
                for kt in range(6):
                    nc.sync.dma_start(out=wq_t[kt][:], in_=wq[kt * P:(kt + 1) * P, :])
                for kt in range(6, NKT):
                    nc.gpsimd.dma_start(out=wq_t[kt][:], in_=wq[kt * P:(kt + 1) * P, :])
                load_xc(1, range(NKT), nc.sync)

                def vproj_pair(c, pair):
                    # two t-tiles of 128 into one [128,512] psum; ACT evicts
                    ps = op_tile()
                    for tl_i in (0, 1):
                        for kt in range(NKT):
                            nc.tensor.matmul(
                                ps[:, tl_i * 256:tl_i * 256 + GDIM],
                                lhsT=xc_tiles[(c, kt)][:, (pair * 2 + tl_i) * P:
                                                       (pair * 2 + tl_i + 1) * P],
                                rhs=wv_t[kt],
                                start=(kt == 0), stop=(kt == NKT - 1))
                    for tl_i in (0, 1):
                        tglob = c * 4 + pair * 2 + tl_i
                        for g in range(NG):
                            nc.scalar.activation(
                                vbig[:, tglob * VSTR + g * (P + 1):
                                     tglob * VSTR + g * (P + 1) + P],
                                ps[:, tl_i * 256 + g * P:tl_i * 256 + (g + 1) * P],
                                AF.Copy)

                def rope_evict(m, ps, c0):
                    # qkT[m][:, c0:c0+512] = ps*cos2 + rot(ps)*sinA   (DVE, f32)
                    t1 = tmp_pool.tile([P, 512], f32, tag="t1", name="t1")
                    t2 = tmp_pool.tile([P, 512], f32, tag="t2", name="t2")
                    h2 = P // 2
                    nc.vector.tensor_mul(t1[:], ps[:], cos_t[:, c0:c0 + 512])
                    nc.vector.tensor_mul(
                        t2[0:h2, :], ps[h2:P, :], sin_t[0:h2, c0:c0 + 512])
                    nc.vector.tensor_mul(
                        t2[h2:P, :], ps[0:h2, :], sin_t[h2:P, c0:c0 + 512])
                    nc.vector.tensor_add(qkT[m][:, c0:c0 + 512], t1[:], t2[:])

                def kq_mgroup(c, m):
                    # m 0..7 -> q head m (wq cols), m 8..9 -> k group (wk cols)
                    ps = op_tile()
                    for kt in range(NKT):
                        if m < NH:
                            lhsT = wq_t[kt][:, m * P:(m + 1) * P]
                        else:
                            lhsT = wk_t[kt][:, (m - NH) * P:(m - NH + 1) * P]
                        nc.tensor.matmul(
                            ps[:], lhsT=lhsT,
                            rhs=xc_tiles[(c, kt)][:],
                            start=(kt == 0), stop=(kt == NKT - 1))
                    rope_evict(m, ps, c * 512)

                for c in range(NCH):
                    if 1 <= c < NCH - 1:
                        load_xc(c + 1, range(NKT), nc.sync)
                    vproj_pair(c, 0)
                    vproj_pair(c, 1)
                    # K first (enables attention right after chunk 3's K)
                    for m in (NH, NH + 1):
                        kq_mgroup(c, m)
                    if c < NCH - 1:
                        for m in range(NH):
                            kq_mgroup(c, m)

                # ---- hybrid: chunk-0 attention, Q3 m-groups as PE filler ----
                q3 = {}

                def q3_unit(h, quarter):
                    # quarter of Q-projection m-group h for chunk 3 (4 MMs)
                    c = NCH - 1
                    if quarter == 0:
                        q3[h] = op_tile()
                    ps = q3[h]
                    for kt in range(quarter * 4, quarter * 4 + 4):
                        nc.tensor.matmul(
                            ps[:], lhsT=wq_t[kt][:, h * P:(h + 1) * P],
                            rhs=xc_tiles[(c, kt)][:],
                            start=(kt == 0), stop=(kt == NKT - 1))
                    if quarter == 3:
                        rope_evict(h, ps, c * 512)

                for h in range(NH):
                    fillers = [None] * 8
                    for q in range(4):
                        fillers[1 + 2 * q] = (lambda hh=h, qq=q: q3_unit(hh, qq))
                    attn_slot(0, h, fillers)

            # ---------------- phase 2: chunks 1-3 + o_proj ------------------
            wo_pool = ctx.enter_context(tc.tile_pool(name="wo", bufs=NDT))
            aT_pool = ctx.enter_context(tc.tile_pool(name="aT", bufs=1))
            osb_pool = ctx.enter_context(tc.tile_pool(name="osb", bufs=6))

            # aTbig[:, dtile*T + qt*128 : +128] = attn_t[qt][:, dtile].T
            aTbig = aT_pool.tile([P, NDT * T], bf16, tag="aT")
            wo_t = []
            for dtile in range(NDT):
                tl = wo_pool.tile([P, D_MODEL], bf16, tag="wo", name=f"wot{dtile}")
                wo_t.append(tl)
            for dtile in range(NDT):
                nc.gpsimd.dma_start(out=wo_t[dtile][:, 0:1024],
                                    in_=wo[dtile * P:(dtile + 1) * P, 0:1024])
            for dtile in range(NDT):
                nc.gpsimd.dma_start(out=wo_t[dtile][:, 1024:2048],
                                    in_=wo[dtile * P:(dtile + 1) * P, 1024:2048])

            scr_state = {"tile": None}

            def transp_unit(qcp, tt, pair):
                # transpose attn_t[qt] dtiles (2*pair, 2*pair+1) into aTbig.
                # One native-bf16 psum tile in the op tag (same 2KB slot size)
                # holds all 8 transposes of a t-tile: plain slices keep the
                # subtile dep tracking precise (a bitcast view here serialized
                # every transpose against the previous region's DVE copy).
                # The transpose's start=True bank-clear cannot disturb any
                # in-flight accumulation since the tile owns its bank.
                qt = qcp * 4 + tt
                if pair == 0:
                    scr_state["tile"] = psum.tile(
                        [P, NDT * P], bf16, tag="op", bufs=2, name="scrt")
                scrt = scr_state["tile"]
                for s2 in range(2):
                    dtile = pair * 2 + s2
                    scr = scrt[:, dtile * P:(dtile + 1) * P]
                    nc.tensor.transpose(
                        scr, attn_t[qt][:, dtile * P:(dtile + 1) * P], identity[:])
                    nc.vector.tensor_copy(
                        aTbig[:, dtile * T + qt * P:dtile * T + (qt + 1) * P], scr)

            op_state = {}

            def oproj_unit(tt, nchn, half):
                # half 0: dtiles 0-3 (start); half 1: dtiles 4-7 (stop+evict)
                if half == 0:
                    op_state[(tt, nchn)] = op_tile()
                ps = op_state[(tt, nchn)]
                for dtile in range(half * 4, half * 4 + 4):
                    nc.tensor.matmul(
                        ps[:],
                        lhsT=aTbig[:, dtile * T + tt * P:dtile * T + (tt + 1) * P],
                        rhs=wo_t[dtile][:, nchn * 512:(nchn + 1) * 512],
                        start=(dtile == 0), stop=(dtile == NDT - 1))
                if half == 1:
                    del op_state[(tt, nchn)]
                    osb = osb_pool.tile([P, 512], bf16, tag="osb", name="osb")
                    # evict on ACT: frees the op bank without queueing behind
                    # the slot's DVE work (normalize + transpose copies)
                    nc.scalar.activation(osb[:], ps[:], AF.Copy)
                    # alternate output queues (sync is idle during phase 2)
                    eng = nc.gpsimd if (tt + nchn) % 2 == 0 else nc.sync
                    eng.dma_start(
                        out=out[tt * P:(tt + 1) * P, nchn * 512:(nchn + 1) * 512],
                        in_=osb[:])

            # filler scheduling: per chunk qc (1..3), slots h=0..7 carry
            # transposes of chunk qc-1 (slot h<4 -> tt=h, 4 pair-units at
            # points 4-7) and o_proj groups of chunk qc-1 (2 units each) from
            # a readiness queue.
            ready_groups = []   # (tt_glob, nchn) ready once tt transposed

            for qc in range(1, NCH):
                qcp = qc - 1
                for h in range(NH):
                    fillers = []
                    trans = []
                    if h < 4:
                        trans = [(lambda t=h, p=p2: transp_unit(qcp, t, p))
                                 for p2 in range(4)]
                    # in the last chunk, hold back one group in slots 2-3
                    # (mid-chunk, where the underfill is too small to trip the
                    # HAM throttle) so ~4 dependency-free groups remain for
                    # the tail to chew on while the attention pipeline drains
                    n_op = min(8 - len(trans), 4)
                    if qc == NCH - 1 and h in (2, 3):
                        n_op = 2
                    opu = []
                    while ready_groups and len(opu) + 2 <= n_op:
                        ttg, nchn = ready_groups.pop(0)
                        opu.append(lambda a=ttg, b=nchn: oproj_unit(a, b, 0))
                        opu.append(lambda a=ttg, b=nchn: oproj_unit(a, b, 1))
                    # op units first (points 0..), transposes at the tail
                    fillers = opu + trans
                    attn_slot(qc, h, fillers)
                    if h < 4:
                        ttg = qcp * 4 + h
                        for nchn in range(NCH):
                            ready_groups.append((ttg, nchn))

            # ---------------- tail: transposes + o_proj of the last chunk ---
            # transposes pace at ~400ns each (LDWEIGHTS + SBUF latency), so
            # interleave every transpose unit with a ready o_proj half-group;
            # the held-back leftover groups prime the queue while the last
            # head's exp/PV/normalize chain drains.
            qcp = NCH - 1
            units = []          # pending o_proj half-group emissions

            def pump():
                if units:
                    units.pop(0)()

            for ttg, nchn in ready_groups:
                for hf in (0, 1):
                    units.append(lambda a=ttg, b=nchn, c=hf: oproj_unit(a, b, c))
            ready_groups = []
            for tt in range(4):
                for p2 in range(4):
                    transp_unit(qcp, tt, p2)
                    pump()
                for nchn in range(NCH):
                    for hf in (0, 1):
                        units.append(lambda a=qcp * 4 + tt, b=nchn, c=hf:
                                     oproj_unit(a, b, c))
            while units:
                pump()

    nc.compile()
    return nc


def make_tables():
    inv_freq = 1.0 / (THETA ** (np.arange(0, HEAD_DIM, 2, dtype=np.float32)
                                / HEAD_DIM))          # [64]
    ang = np.arange(T, dtype=np.float32)[:, None] * inv_freq[None, :]  # [T, 64]
    cos = np.cos(ang).T.astype(np.float32)            # [64, T]
    sin = np.sin(ang).T.astype(np.float32)
    cos2 = np.concatenate([cos, cos], axis=0)         # [128, T]
    sinA = np.concatenate([-sin, sin], axis=0)        # [128, T]
    return (np.ascontiguousarray(cos2).astype(np.float16),
            np.ascontiguousarray(sinA).astype(np.float16))


def make_in_maps(x, W_qkv, W_o, padding_mask, masked):
    cos2_v, sinA_v = make_tables()
    in_maps = []
    for c in range(N_CORES):
        b, half = c // 2, c % 2
        q0 = half * QDIM
        k0 = NUM_HEADS * HEAD_DIM + half * GDIM
        v0 = NUM_HEADS * HEAD_DIM + QUERY_GROUPS * HEAD_DIM + half * GDIM
        wvk_v = np.concatenate(
            [W_qkv[:, v0:v0 + GDIM], W_qkv[:, k0:k0 + GDIM]], axis=1)
        m = {
            "xt": np.ascontiguousarray(x[b].T).astype(BF16),
            "wq": np.ascontiguousarray(W_qkv[:, q0:q0 + QDIM]).astype(BF16),
            "wvk": np.ascontiguousarray(wvk_v).astype(BF16),
            "wo": np.ascontiguousarray(W_o[half * QDIM:(half + 1) * QDIM, :]).astype(BF16),
            "cos2": cos2_v, "sinA": sinA_v,
        }
        if masked:
            bias = np.where(padding_mask[b], 0.0, -1e30).astype(np.float32)  # [T]
            m["maskcol"] = np.ascontiguousarray(
                bias.reshape(NTT, P).T).astype(np.float32)
        in_maps.append(m)
    return in_maps


_nc_cache = {}


def kernel(x, W_qkv, W_o, padding_mask, trace=False):
    from concourse.bass_utils import run_bass_kernel_spmd

    x = np.asarray(x)
    W_qkv = np.asarray(W_qkv)
    W_o = np.asarray(W_o)
    padding_mask = np.asarray(padding_mask)
    masked = not bool(padding_mask.all())

    if masked not in _nc_cache:
        _nc_cache[masked] = build_nc(masked)
    nc = _nc_cache[masked]

    in_maps = make_in_maps(x, W_qkv, W_o, padding_mask, masked)
    res = run_bass_kernel_spmd(
        nc, in_maps, core_ids=list(range(N_CORES)),
        trace=trace, trace_cores=[0] if trace else None)

    out = np.empty((B, T, D_MODEL), np.float32)
    for b in range(B):
        out[b] = (res.results[2 * b]["out"].astype(np.float32)
                  + res.results[2 * b + 1]["out"].astype(np.float32))
    kernel.last_exec_time_ns = res.exec_time_ns
    kernel.last_results = res
    return out


# revision 35
# speedup vs baseline: 1.0058x; 1.0034x over previous
"""Trainium2 Bass kernel for GQA attention (B=4, T=2048, D=2048, 16 heads / 4 kv groups, RoPE).

Sharding: 8 cores = 4 batches x 2 head-halves. Core c handles batch c//2 and
heads (c%2)*8..+8 with kv groups (c%2)*2..+2.

Structure (vs v1 baseline, 659us -> ~583us):
  - phase 1: per chunk [V-proj pairs -> K-proj -> Q-proj] channel-major with
    RoPE fused into psum eviction (DVE); V eviction on ACT; 1/sqrt(d) folded
    into the exp activation scale so only 2 unscaled fp16 rope tables load.
  - chunk 3 defers its Q-projection: the 8 Q m-groups are interleaved as PE
    filler into chunk-0's attention head slots (which are otherwise exp/ACT
    bound), after K3/V3 complete.
  - attention: per (qc,h): S^T tiles [k=128, q=512x2] -> exp(scale*s) on ACT
    -> PV via ones-augmented v (denominator in psum col 128), software
    pipelined one ktp ahead, with o_proj matmuls + PE transposes of the
    previous chunk interleaved as per-ktp filler so the PE never waits on ACT;
    each head's normalize emitted at its slot end to free the pv banks early.
  - psum: "mm" 2x[128,1024] scores, "op" 2x[128,512] proj/oproj/transpose-
    scratch, "pv" 1x[128,1024] packed pv regions = exactly 8 banks (a psum
    accumulation group's start=True clears its whole bank's has_written bits,
    so regions sharing a bank share one accumulation group).
  - DMA ordered for the per-core HBM limit: xc0+wvk first (vproj gate) on
    sync/gpsimd, then tables/wq, then later chunks; outputs alternate
    sync/gpsimd queues; the scalar (ACT) stream carries no DMAs.
All matmuls bf16 with fp32 PSUM accumulation; bf16 device output, host sums
the two half-core partials in fp32.
"""

import numpy as np
import ml_dtypes

BF16 = ml_dtypes.bfloat16

D_MODEL = 2048
NUM_HEADS = 16
QUERY_GROUPS = 4
HEAD_DIM = 128
B = 4
T = 2048
THETA = 10000.0
SCALE = 0.08838834764831845
N_CORES = 8

P = 128
NH = NUM_HEADS // 2          # 8 q heads per core
NG = QUERY_GROUPS // 2       # 2 kv groups per core
QDIM = NH * HEAD_DIM         # 1024
GDIM = NG * HEAD_DIM         # 256
NKT = D_MODEL // P           # 16 contraction tiles over d_model
NTT = T // P                 # 16 tiles over sequence
NCH = T // 512               # 4 chunks of 512 over sequence
NDT = QDIM // P              # 8 head/dim tiles per core

# f32 col offsets of the 4 pv regions in the pv psum tile. j0/j1 share bank A
# (cols 0-511), j2/j3 share bank B (cols 512-1023); each bank's two regions
# form ONE psum accumulation group (start only on the bank's first matmul)
# because start=True clears the whole bank's has_written bits.
OFFJ = [0, 132, 512, 644]
SCRW = 64                    # f32 cols per bf16 [128,128] transpose scratch
VSTR = 2 * (P + 1)           # 258 bf16 cols per t-tile block in vbig


def build_nc(masked: bool):
    import concourse.bacc as bacc
    import concourse.tile as tile
    import concourse.mybir as mybir
    from concourse.masks import make_identity
    from contextlib import ExitStack

    dt = mybir.dt
    f32 = dt.float32
    bf16 = dt.bfloat16
    AF = mybir.ActivationFunctionType

    nc = bacc.Bacc("TRN2", target_bir_lowering=False, debug=False, num_devices=N_CORES)

    xt = nc.dram_tensor("xt", [D_MODEL, T], bf16, kind="ExternalInput")
    wq = nc.dram_tensor("wq", [D_MODEL, QDIM], bf16, kind="ExternalInput")
    # wv|wk packed so the tiles have 1KB lines (512B-line DMAs run ~2x slower)
    wvk = nc.dram_tensor("wvk", [D_MODEL, 2 * GDIM], bf16, kind="ExternalInput")
    wo = nc.dram_tensor("wo", [QDIM, D_MODEL], bf16, kind="ExternalInput")
    cos2 = nc.dram_tensor("cos2", [P, T], dt.float16, kind="ExternalInput")
    sinA = nc.dram_tensor("sinA", [P, T], dt.float16, kind="ExternalInput")
    if masked:
        maskcol = nc.dram_tensor("maskcol", [P, NTT], f32, kind="ExternalInput")
    out = nc.dram_tensor("out", [T, D_MODEL], bf16, kind="ExternalOutput")

    with tile.TileContext(nc) as tc:
        with ExitStack() as ctx:
            psum = ctx.enter_context(tc.tile_pool(name="ps", bufs=1, space="PSUM"))
            constp = ctx.enter_context(tc.tile_pool(name="const", bufs=1))
            qkT_pool = ctx.enter_context(tc.tile_pool(name="qkT", bufs=NH + NG))
            vbig_pool = ctx.enter_context(tc.tile_pool(name="vbig", bufs=1))
            attn_pool = ctx.enter_context(tc.tile_pool(name="attn", bufs=NTT))
            pt_pool = ctx.enter_context(tc.tile_pool(name="pt", bufs=4))
            rc_pool = ctx.enter_context(tc.tile_pool(name="rc", bufs=8))

            def mm_tile():
                return psum.tile([P, 1024], f32, tag="mm", bufs=2, name="mmt")

            def op_tile():
                return psum.tile([P, 512], f32, tag="op", bufs=2, name="opt")

            def pv_tile():
                return psum.tile([P, 1024], f32, tag="pv", bufs=1, name="pvt")

            identity = constp.tile([P, P], bf16, tag="identity")
            make_identity(nc, identity[:])
            dummy = constp.tile([P, 256], bf16, tag="dummy")
            nc.vector.memset(dummy[:], 0.0)
            if masked:
                maskcol_t = constp.tile([P, NTT], f32, tag="maskcol")
                nc.gpsimd.dma_start(out=maskcol_t[:], in_=maskcol[:, :])

            # persistent bf16 tensors
            # qkT[0..7] = q heads, qkT[8..9] = k groups; [d=128, T] channel-major
            qkT = [qkT_pool.tile([P, T], bf16, tag="qkT", name=f"qkT{i}")
                   for i in range(NH + NG)]
            # vbig: per t-tile block of 258 cols: [v_g0(128) | ones | v_g1(128) | ones]
            vbig = vbig_pool.tile([P, NTT * VSTR], bf16, tag="vbig")
            for i in range(NTT):
                for g in range(NG):
                    c = i * VSTR + g * (P + 1) + P
                    nc.vector.memset(vbig[:, c:c + 1], 1.0)
            attn_t = [attn_pool.tile([P, QDIM], bf16, tag="attn", name=f"attn{i}")
                      for i in range(NTT)]

            # ---------------- warmup (HAM + cover initial DMA latency) ------
            warm = op_tile()
            for i in range(48):
                nc.tensor.matmul(warm[:, 0:256], lhsT=identity[:], rhs=dummy[:],
                                 start=(i == 0), stop=(i == 47))
            wsink = constp.tile([P, 16], f32, tag="wsink")
            nc.vector.tensor_copy(wsink[:], warm[:, 0:16])

            # ---------------- shared attention machinery -------------------
            def normalize(qc, h, pvs_prev):
                # attn_t[qt][:, h*128:+128] = pvs[:, j] / denom  (DVE)
                for j in range(4):
                    qt = qc * 4 + j
                    rc = rc_pool.tile([P, 1], f32, tag="rc", name="rc")
                    nc.vector.reciprocal(
                        rc[:], pvs_prev[:, OFFJ[j] + P:OFFJ[j] + P + 1])
                    nc.vector.tensor_scalar_mul(
                        attn_t[qt][:, h * P:(h + 1) * P],
                        pvs_prev[:, OFFJ[j]:OFFJ[j] + P], rc[:])

            def attn_slot(qc, h, fillers):
                """One head slot: scores+exp+PV pipelined 1 ktp deep, with
                `fillers` (list of <=8 callables of ~0.5-1us PE work each)
                emitted at the 8 interleave points."""
                g = h // 4
                kT = qkT[NH + g]
                qT = qkT[h]
                c0 = qc * 512

                pvs = pv_tile()

                ps_l = [None] * 8
                pt_l = [None] * 8

                def emit_mms(ktp):
                    ps = mm_tile()
                    ps_l[ktp] = ps
                    for s in range(2):
                        kt = ktp * 2 + s
                        nc.tensor.matmul(
                            ps[:, s * 512:(s + 1) * 512],
                            lhsT=kT[:, kt * P:(kt + 1) * P],
                            rhs=qT[:, c0:c0 + 512],
                            start=True, stop=True)
                    pt = pt_pool.tile([P, 1024], bf16, tag="pt", name="pt")
                    pt_l[ktp] = pt
                    if masked:
                        for s in range(2):
                            kt = ktp * 2 + s
                            nc.scalar.activation(
                                pt[:, s * 512:(s + 1) * 512],
                                ps[:, s * 512:(s + 1) * 512],
                                AF.Exp, bias=maskcol_t[:, kt:kt + 1], scale=SCALE)
                    else:
                        nc.scalar.activation(pt[:], ps[:], AF.Exp, scale=SCALE)

                def emit_pv(ktp):
                    pt = pt_l[ktp]
                    for s in range(2):
                        kt = ktp * 2 + s
                        for j in range(4):
                            # j0/j1 (bank A) and j2/j3 (bank B) each form one
                            # accumulation group: start only on the bank's
                            # first matmul, stop on its last.
                            nc.tensor.matmul(
                                pvs[:, OFFJ[j]:OFFJ[j] + P + 1],
                                lhsT=pt[:, s * 512 + j * P:s * 512 + (j + 1) * P],
                                rhs=vbig[:, kt * VSTR + g * (P + 1):
                                         kt * VSTR + (g + 1) * (P + 1)],
                                start=(kt == 0 and j % 2 == 0),
                                stop=(kt == NTT - 1 and j % 2 == 1),
                                skip_group_check=True)

                emit_mms(0)
                for ktp in range(8):
                    if ktp < 7:
                        emit_mms(ktp + 1)
                    if ktp < len(fillers) and fillers[ktp] is not None:
                        fillers[ktp]()
                    emit_pv(ktp)

                # emit this head's normalize now so it sits ahead of the next
                # slot's filler casts in the DVE queue: the pv banks then free
                # ~1us into the next slot, before its first PV matmul.
                normalize(qc, h, pvs)

            # ---------------- phase 1: projections + rope -------------------
            with ExitStack() as ph1:
                wq_pool = ph1.enter_context(tc.tile_pool(name="wq", bufs=NKT))
                wvk_pool = ph1.enter_context(tc.tile_pool(name="wvk", bufs=NKT))
                xc_pool = ph1.enter_context(tc.tile_pool(name="xc", bufs=32))
                tab_pool = ph1.enter_context(tc.tile_pool(name="tab", bufs=1))
                tmp_pool = ph1.enter_context(tc.tile_pool(name="rtmp", bufs=2))

                # --- DMA emission: sync + gpsimd queues only, so the scalar
                # (ACT) instruction stream stays free for psum evictions ---
                xc_tiles = {}

                def load_xc(nch, kts, eng):
                    c0 = nch * 512
                    for kt in kts:
                        tl = xc_pool.tile([P, 512], bf16, tag="xc", name=f"xc{nch}_{kt}")
                        eng.dma_start(out=tl[:], in_=xt[kt * P:(kt + 1) * P, c0:c0 + 512])
                        xc_tiles[(nch, kt)] = tl

                # per-core HBM (~358 GB/s) is the startup wall: put ONLY the
                # vproj gate (xc0 on sync, wvk on gpsimd) in the first window,
                # then tables, then wq, then xc1.
                load_xc(0, range(NKT), nc.sync)
                wvk_t = []
                for kt in range(NKT):
                    tl = wvk_pool.tile([P, 2 * GDIM], bf16, tag="wvk", name=f"wvkt{kt}")
                    nc.gpsimd.dma_start(out=tl[:], in_=wvk[kt * P:(kt + 1) * P, :])
                    wvk_t.append(tl)
                wv_t = [tl[:, 0:GDIM] for tl in wvk_t]
                wk_t = [tl[:, GDIM:2 * GDIM] for tl in wvk_t]
                cos_t = tab_pool.tile([P, T], dt.float16, tag="cos", name="cos_t")
                nc.gpsimd.dma_start(out=cos_t[:], in_=cos2[:, :])
                sin_t = tab_pool.tile([P, T], dt.float16, tag="sin", name="sin_t")
                nc.gpsimd.dma_start(out=sin_t[:], in_=sinA[:, :])
                wq_t = [wq_pool.tile([P, QDIM], bf16, tag="wq", name=f"wqt{kt}")
                        for kt in range(NKT)]
                for kt in range(6):
                    nc.sync.dma_start(out=wq_t[kt][:], in_=wq[kt * P:(kt + 1) * P, :])
                for kt in range(6, NKT):
                    nc.gpsimd.dma_start(out=wq_t[kt][:], in_=wq[kt * P:(kt + 1) * P, :])
                load_xc(1, range(NKT), nc.sync)

                def vproj_pair(c, pair):
                    # two t-tiles of 128 into one [128,512] psum; ACT evicts
                    ps = op_tile()
                    for tl_i in (0, 1):
                        for kt in range(NKT):
                            nc.tensor.matmul(
                                ps[:, tl_i * 256:tl_i * 256 + GDIM],
                                lhsT=xc_tiles[(c, kt)][:, (pair * 2 + tl_i) * P:
                                                       (pair * 2 + tl_i + 1) * P],
                                rhs=wv_t[kt],
                                start=(kt == 0), stop=(kt == NKT - 1))
                    for tl_i in (0, 1):
                        tglob = c * 4 + pair * 2 + tl_i
                        for g in range(NG):
                            nc.scalar.activation(
                                vbig[:, tglob * VSTR + g * (P + 1):
                                     tglob * VSTR + g * (P + 1) + P],
                                ps[:, tl_i * 256 + g * P:tl_i * 256 + (g + 1) * P],
                                AF.Copy)

                def rope_evict(m, ps, c0):
                    # qkT[m][:, c0:c0+512] = ps*cos2 + rot(ps)*sinA   (DVE, f32)
                    t1 = tmp_pool.tile([P, 512], f32, tag="t1", name="t1")
                    t2 = tmp_pool.tile([P, 512], f32, tag="t2", name="t2")
                    h2 = P // 2
                    nc.vector.tensor_mul(t1[:], ps[:], cos_t[:, c0:c0 + 512])
                    nc.vector.tensor_mul(
                        t2[0:h2, :], ps[h2:P, :], sin_t[0:h2, c0:c0 + 512])
                    nc.vector.tensor_mul(
                        t2[h2:P, :], ps[0:h2, :], sin_t[h2:P, c0:c0 + 512])
                    nc.vector.tensor_add(qkT[m][:, c0:c0 + 512], t1[:], t2[:])

                def kq_mgroup(c, m):
                    # m 0..7 -> q head m (wq cols), m 8..9 -> k group (wk cols)
                    ps = op_tile()
                    for kt in range(NKT):
                        if m < NH:
                            lhsT = wq_t[kt][:, m * P:(m + 1) * P]
                        else:
                            lhsT = wk_t[kt][:, (m - NH) * P:(m - NH + 1) * P]
                        nc.tensor.matmul(
                            ps[:], lhsT=lhsT,
                            rhs=xc_tiles[(c, kt)][:],
                            start=(kt == 0), stop=(kt == NKT - 1))
                    rope_evict(m, ps, c * 512)

                for c in range(NCH):
                    if 1 <= c < NCH - 1:
                        load_xc(c + 1, range(NKT), nc.sync)
                    vproj_pair(c, 0)
                    vproj_pair(c, 1)
                    # K first (enables attention right after chunk 3's K)
                    for m in (NH, NH + 1):
                        kq_mgroup(c, m)
                    if c < NCH - 1:
                        for m in range(NH):
                            kq_mgroup(c, m)

                # ---- hybrid: chunk-0 attention, Q3 m-groups as PE filler ----
                q3 = {}

                def q3_unit(h, quarter):
                    # quarter of Q-projection m-group h for chunk 3 (4 MMs)
                    c = NCH - 1
                    if quarter == 0:
                        q3[h] = op_tile()
                    ps = q3[h]
                    for kt in range(quarter * 4, quarter * 4 + 4):
                        nc.tensor.matmul(
                            ps[:], lhsT=wq_t[kt][:, h * P:(h + 1) * P],
                            rhs=xc_tiles[(c, kt)][:],
                            start=(kt == 0), stop=(kt == NKT - 1))
                    if quarter == 3:
                        rope_evict(h, ps, c * 512)

                for h in range(NH):
                    fillers = [None] * 8
                    for q in range(4):
                        fillers[1 + 2 * q] = (lambda hh=h, qq=q: q3_unit(hh, qq))
                    attn_slot(0, h, fillers)

            # ---------------- phase 2: chunks 1-3 + o_proj ------------------
            wo_pool = ctx.enter_context(tc.tile_pool(name="wo", bufs=NDT))
            aT_pool = ctx.enter_context(tc.tile_pool(name="aT", bufs=1))
            osb_pool = ctx.enter_context(tc.tile_pool(name="osb", bufs=6))

            # aTbig[:, dtile*T + qt*128 : +128] = attn_t[qt][:, dtile].T
            aTbig = aT_pool.tile([P, NDT * T], bf16, tag="aT")
            wo_t = []
            for dtile in range(NDT):
                tl = wo_pool.tile([P, D_MODEL], bf16, tag="wo", name=f"wot{dtile}")
                wo_t.append(tl)
            for dtile in range(NDT):
                nc.gpsimd.dma_start(out=wo_t[dtile][:, 0:1024],
                                    in_=wo[dtile * P:(dtile + 1) * P, 0:1024])
            for dtile in range(NDT):
                nc.gpsimd.dma_start(out=wo_t[dtile][:, 1024:2048],
                                    in_=wo[dtile * P:(dtile + 1) * P, 1024:2048])

            scr_state = {"tile": None}

            def transp_unit(qcp, tt, pair):
                # transpose attn_t[qt] dtiles (2*pair, 2*pair+1) into aTbig.
                # One native-bf16 psum tile in the op tag (same 2KB slot size)
                # holds all 8 transposes of a t-tile: plain slices keep the
                # subtile dep tracking precise (a bitcast view here serialized
                # every transpose against the previous region's DVE copy).
                # The transpose's start=True bank-clear cannot disturb any
                # in-flight accumulation since the tile owns its bank.
                qt = qcp * 4 + tt
                if pair == 0:
                    scr_state["tile"] = psum.tile(
                        [P, NDT * P], bf16, tag="op", bufs=2, name="scrt")
                scrt = scr_state["tile"]
                for s2 in range(2):
                    dtile = pair * 2 + s2
                    scr = scrt[:, dtile * P:(dtile + 1) * P]
                    nc.tensor.transpose(
                        scr, attn_t[qt][:, dtile * P:(dtile + 1) * P], identity[:])
                    nc.vector.tensor_copy(
                        aTbig[:, dtile * T + qt * P:dtile * T + (qt + 1) * P], scr)

            op_state = {}

            def oproj_unit(tt, nchn, half):
                # half 0: dtiles 0-3 (start); half 1: dtiles 4-7 (stop+evict)
                if half == 0:
                    op_state[(tt, nchn)] = op_tile()
                ps = op_state[(tt, nchn)]
                for dtile in range(half * 4, half * 4 + 4):
                    nc.tensor.matmul(
                        ps[:],
                        lhsT=aTbig[:, dtile * T + tt * P:dtile * T + (tt + 1) * P],
                        rhs=wo_t[dtile][:, nchn * 512:(nchn + 1) * 512],
                        start=(dtile == 0), stop=(dtile == NDT - 1))
                if half == 1:
                    del op_state[(tt, nchn)]
                    osb = osb_pool.tile([P, 512], bf16, tag="osb", name="osb")
                    # evict on ACT: frees the op bank without queueing behind
                    # the slot's DVE work (normalize + transpose copies)
                    nc.scalar.activation(osb[:], ps[:], AF.Copy)
                    # alternate output queues (sync is idle during phase 2)
                    eng = nc.gpsimd if (tt + nchn) % 2 == 0 else nc.sync
                    eng.dma_start(
                        out=out[tt * P:(tt + 1) * P, nchn * 512:(nchn + 1) * 512],
                        in_=osb[:])

            # filler scheduling: per chunk qc (1..3), slots h=0..7 carry
            # transposes of chunk qc-1 (slot h<4 -> tt=h, 4 pair-units at
            # points 4-7) and o_proj groups of chunk qc-1 (2 units each) from
            # a readiness queue.
            ready_groups = []   # (tt_glob, nchn) ready once tt transposed

            for qc in range(1, NCH):
                qcp = qc - 1
                for h in range(NH):
                    fillers = []
                    trans = []
                    if h < 4:
                        trans = [(lambda t=h, p=p2: transp_unit(qcp, t, p))
                                 for p2 in range(4)]
                    n_op = min(8 - len(trans), 4)
                    opu = []
                    while ready_groups and len(opu) + 2 <= n_op:
                        ttg, nchn = ready_groups.pop(0)
                        opu.append(lambda a=ttg, b=nchn: oproj_unit(a, b, 0))
                        opu.append(lambda a=ttg, b=nchn: oproj_unit(a, b, 1))
                    # op units first (points 0..), transposes at the tail
                    fillers = opu + trans
                    attn_slot(qc, h, fillers)
                    if h < 4:
                        ttg = qcp * 4 + h
                        for nchn in range(NCH):
                            ready_groups.append((ttg, nchn))

            # ---------------- tail: transposes + o_proj of the last chunk ---
            # transposes pace at ~400ns each (LDWEIGHTS + SBUF latency), so
            # interleave every transpose unit with a ready o_proj half-group;
            # the held-back leftover groups prime the queue while the last
            # head's exp/PV/normalize chain drains.
            qcp = NCH - 1
            units = []          # pending o_proj half-group emissions

            def pump():
                if units:
                    units.pop(0)()

            for ttg, nchn in ready_groups:
                for hf in (0, 1):
                    units.append(lambda a=ttg, b=nchn, c=hf: oproj_unit(a, b, c))
            ready_groups = []
            for tt in range(4):
                for p2 in range(4):
                    transp_unit(qcp, tt, p2)
                    pump()
                for nchn in range(NCH):
                    for hf in (0, 1):
                        units.append(lambda a=qcp * 4 + tt, b=nchn, c=hf:
                                     oproj_unit(a, b, c))
            while units:
                pump()

    nc.compile()
    return nc


def make_tables():
    inv_freq = 1.0 / (THETA ** (np.arange(0, HEAD_DIM, 2, dtype=np.float32)
                                / HEAD_DIM))          # [64]
    ang = np.arange(T, dtype=np.float32)[:, None] * inv_freq[None, :]  # [T, 64]
    cos = np.cos(ang).T.astype(np.float32)            # [64, T]
    sin = np.sin(ang).T.astype(np.float32)
    cos2 = np.concatenate([cos, cos], axis=0)         # [128, T]
    sinA = np.concatenate([-sin, sin], axis=0)        # [128, T]
    return (np.ascontiguousarray(cos2).astype(np.float16),
            np.ascontiguousarray(sinA).astype(np.float16))


def make_in_maps(x, W_qkv, W_o, padding_mask, masked):
    cos2_v, sinA_v = make_tables()
    in_maps = []
    for c in range(N_CORES):
        b, half = c // 2, c % 2
        q0 = half * QDIM
        k0 = NUM_HEADS * HEAD_DIM + half * GDIM
        v0 = NUM_HEADS * HEAD_DIM + QUERY_GROUPS * HEAD_DIM + half * GDIM
        wvk_v = np.concatenate(
            [W_qkv[:, v0:v0 + GDIM], W_qkv[:, k0:k0 + GDIM]], axis=1)
        m = {
            "xt": np.ascontiguousarray(x[b].T).astype(BF16),
            "wq": np.ascontiguousarray(W_qkv[:, q0:q0 + QDIM]).astype(BF16),
            "wvk": np.ascontiguousarray(wvk_v).astype(BF16),
            "wo": np.ascontiguousarray(W_o[half * QDIM:(half + 1) * QDIM, :]).astype(BF16),
            "cos2": cos2_v, "sinA": sinA_v,
        }
        if masked:
            bias = np.where(padding_mask[b], 0.0, -1e30).astype(np.float32)  # [T]
            m["maskcol"] = np.ascontiguousarray(
                bias.reshape(NTT, P).T).astype(np.float32)
        in_maps.append(m)
    return in_maps


_nc_cache = {}


def kernel(x, W_qkv, W_o, padding_mask, trace=False):
    from concourse.bass_utils import run_bass_kernel_spmd

    x = np.asarray(x)
    W_qkv = np.asarray(W_qkv)
    W_o = np.asarray(W_o)
    padding_mask = np.asarray(padding_mask)
    masked = not bool(padding_mask.all())

    if masked not in _nc_cache:
        _nc_cache[masked] = build_nc(masked)
    nc = _nc_cache[masked]

    in_maps = make_in_maps(x, W_qkv, W_o, padding_mask, masked)
    res = run_bass_kernel_spmd(
        nc, in_maps, core_ids=list(range(N_CORES)),
        trace=trace, trace_cores=[0] if trace else None)

    out = np.empty((B, T, D_MODEL), np.float32)
    for b in range(B):
        out[b] = (res.results[2 * b]["out"].astype(np.float32)
                  + res.results[2 * b + 1]["out"].astype(np.float32))
    kernel.last_exec_time_ns = res.exec_time_ns
    kernel.last_results = res
    return out


# revision 36
# speedup vs baseline: 1.0083x; 1.0024x over previous
"""Trainium2 Bass kernel for GQA attention (B=4, T=2048, D=2048, 16 heads / 4 kv groups, RoPE).

Sharding: 8 cores = 4 batches x 2 head-halves. Core c handles batch c//2 and
heads (c%2)*8..+8 with kv groups (c%2)*2..+2.

Structure (vs v1 baseline, 659us -> ~583us):
  - phase 1: per chunk [V-proj pairs -> K-proj -> Q-proj] channel-major with
    RoPE fused into psum eviction (DVE); V eviction on ACT; 1/sqrt(d) folded
    into the exp activation scale so only 2 unscaled fp16 rope tables load.
  - chunk 3 defers its Q-projection: the 8 Q m-groups are interleaved as PE
    filler into chunk-0's attention head slots (which are otherwise exp/ACT
    bound), after K3/V3 complete.
  - attention: per (qc,h): S^T tiles [k=128, q=512x2] -> exp(scale*s) on ACT
    -> PV via ones-augmented v (denominator in psum col 128), software
    pipelined one ktp ahead, with o_proj matmuls + PE transposes of the
    previous chunk interleaved as per-ktp filler so the PE never waits on ACT;
    each head's normalize emitted at its slot end to free the pv banks early.
  - psum: "mm" 2x[128,1024] scores, "op" 2x[128,512] proj/oproj/transpose-
    scratch, "pv" 1x[128,1024] packed pv regions = exactly 8 banks (a psum
    accumulation group's start=True clears its whole bank's has_written bits,
    so regions sharing a bank share one accumulation group).
  - DMA ordered for the per-core HBM limit: xc0+wvk first (vproj gate) on
    sync/gpsimd, then tables/wq, then later chunks; outputs alternate
    sync/gpsimd queues; the scalar (ACT) stream carries no DMAs.
All matmuls bf16 with fp32 PSUM accumulation; bf16 device output, host sums
the two half-core partials in fp32.
"""

import numpy as np
import ml_dtypes

BF16 = ml_dtypes.bfloat16

D_MODEL = 2048
NUM_HEADS = 16
QUERY_GROUPS = 4
HEAD_DIM = 128
B = 4
T = 2048
THETA = 10000.0
SCALE = 0.08838834764831845
N_CORES = 8

P = 128
NH = NUM_HEADS // 2          # 8 q heads per core
NG = QUERY_GROUPS // 2       # 2 kv groups per core
QDIM = NH * HEAD_DIM         # 1024
GDIM = NG * HEAD_DIM         # 256
NKT = D_MODEL // P           # 16 contraction tiles over d_model
NTT = T // P                 # 16 tiles over sequence
NCH = T // 512               # 4 chunks of 512 over sequence
NDT = QDIM // P              # 8 head/dim tiles per core

# f32 col offsets of the 4 pv regions in the pv psum tile. j0/j1 share bank A
# (cols 0-511), j2/j3 share bank B (cols 512-1023); each bank's two regions
# form ONE psum accumulation group (start only on the bank's first matmul)
# because start=True clears the whole bank's has_written bits.
OFFJ = [0, 132, 512, 644]
SCRW = 64                    # f32 cols per bf16 [128,128] transpose scratch
VSTR = 2 * (P + 1)           # 258 bf16 cols per t-tile block in vbig


def build_nc(masked: bool):
    import concourse.bacc as bacc
    import concourse.tile as tile
    import concourse.mybir as mybir
    from concourse.masks import make_identity
    from contextlib import ExitStack

    dt = mybir.dt
    f32 = dt.float32
    bf16 = dt.bfloat16
    AF = mybir.ActivationFunctionType

    nc = bacc.Bacc("TRN2", target_bir_lowering=False, debug=False, num_devices=N_CORES)

    xt = nc.dram_tensor("xt", [D_MODEL, T], bf16, kind="ExternalInput")
    wq = nc.dram_tensor("wq", [D_MODEL, QDIM], bf16, kind="ExternalInput")
    # wv|wk packed so the tiles have 1KB lines (512B-line DMAs run ~2x slower)
    wvk = nc.dram_tensor("wvk", [D_MODEL, 2 * GDIM], bf16, kind="ExternalInput")
    wo = nc.dram_tensor("wo", [QDIM, D_MODEL], bf16, kind="ExternalInput")
    cos2 = nc.dram_tensor("cos2", [P, T], dt.float16, kind="ExternalInput")
    sinA = nc.dram_tensor("sinA", [P, T], dt.float16, kind="ExternalInput")
    if masked:
        maskcol = nc.dram_tensor("maskcol", [P, NTT], f32, kind="ExternalInput")
    out = nc.dram_tensor("out", [T, D_MODEL], bf16, kind="ExternalOutput")

    with tile.TileContext(nc) as tc:
        with ExitStack() as ctx:
            psum = ctx.enter_context(tc.tile_pool(name="ps", bufs=1, space="PSUM"))
            constp = ctx.enter_context(tc.tile_pool(name="const", bufs=1))
            qkT_pool = ctx.enter_context(tc.tile_pool(name="qkT", bufs=NH + NG))
            vbig_pool = ctx.enter_context(tc.tile_pool(name="vbig", bufs=1))
            attn_pool = ctx.enter_context(tc.tile_pool(name="attn", bufs=NTT))
            pt_pool = ctx.enter_context(tc.tile_pool(name="pt", bufs=4))
            rc_pool = ctx.enter_context(tc.tile_pool(name="rc", bufs=8))

            def mm_tile():
                return psum.tile([P, 1024], f32, tag="mm", bufs=2, name="mmt")

            def op_tile():
                return psum.tile([P, 512], f32, tag="op", bufs=2, name="opt")

            def pv_tile():
                return psum.tile([P, 1024], f32, tag="pv", bufs=1, name="pvt")

            identity = constp.tile([P, P], bf16, tag="identity")
            make_identity(nc, identity[:])
            dummy = constp.tile([P, 256], bf16, tag="dummy")
            nc.vector.memset(dummy[:], 0.0)
            if masked:
                maskcol_t = constp.tile([P, NTT], f32, tag="maskcol")
                nc.gpsimd.dma_start(out=maskcol_t[:], in_=maskcol[:, :])

            # persistent bf16 tensors
            # qkT[0..7] = q heads, qkT[8..9] = k groups; [d=128, T] channel-major
            qkT = [qkT_pool.tile([P, T], bf16, tag="qkT", name=f"qkT{i}")
                   for i in range(NH + NG)]
            # vbig: per t-tile block of 258 cols: [v_g0(128) | ones | v_g1(128) | ones]
            vbig = vbig_pool.tile([P, NTT * VSTR], bf16, tag="vbig")
            for i in range(NTT):
                for g in range(NG):
                    c = i * VSTR + g * (P + 1) + P
                    nc.vector.memset(vbig[:, c:c + 1], 1.0)
            attn_t = [attn_pool.tile([P, QDIM], bf16, tag="attn", name=f"attn{i}")
                      for i in range(NTT)]

            # ---------------- warmup (HAM + cover initial DMA latency) ------
            warm = op_tile()
            for i in range(48):
                nc.tensor.matmul(warm[:, 0:256], lhsT=identity[:], rhs=dummy[:],
                                 start=(i == 0), stop=(i == 47))
            wsink = constp.tile([P, 16], f32, tag="wsink")
            nc.vector.tensor_copy(wsink[:], warm[:, 0:16])

            # ---------------- shared attention machinery -------------------
            def normalize(qc, h, pvs_prev):
                # attn_t[qt][:, h*128:+128] = pvs[:, j] / denom  (DVE)
                for j in range(4):
                    qt = qc * 4 + j
                    rc = rc_pool.tile([P, 1], f32, tag="rc", name="rc")
                    nc.vector.reciprocal(
                        rc[:], pvs_prev[:, OFFJ[j] + P:OFFJ[j] + P + 1])
                    nc.vector.tensor_scalar_mul(
                        attn_t[qt][:, h * P:(h + 1) * P],
                        pvs_prev[:, OFFJ[j]:OFFJ[j] + P], rc[:])

            def attn_slot(qc, h, fillers):
                """One head slot: scores+exp+PV pipelined 1 ktp deep, with
                `fillers` (list of <=8 callables of ~0.5-1us PE work each)
                emitted at the 8 interleave points."""
                g = h // 4
                kT = qkT[NH + g]
                qT = qkT[h]
                c0 = qc * 512

                pvs = pv_tile()

                ps_l = [None] * 8
                pt_l = [None] * 8

                def emit_mms(ktp):
                    ps = mm_tile()
                    ps_l[ktp] = ps
                    for s in range(2):
                        kt = ktp * 2 + s
                        nc.tensor.matmul(
                            ps[:, s * 512:(s + 1) * 512],
                            lhsT=kT[:, kt * P:(kt + 1) * P],
                            rhs=qT[:, c0:c0 + 512],
                            start=True, stop=True)
                    pt = pt_pool.tile([P, 1024], bf16, tag="pt", name="pt")
                    pt_l[ktp] = pt
                    if masked:
                        for s in range(2):
                            kt = ktp * 2 + s
                            nc.scalar.activation(
                                pt[:, s * 512:(s + 1) * 512],
                                ps[:, s * 512:(s + 1) * 512],
                                AF.Exp, bias=maskcol_t[:, kt:kt + 1], scale=SCALE)
                    else:
                        nc.scalar.activation(pt[:], ps[:], AF.Exp, scale=SCALE)

                def emit_pv(ktp):
                    pt = pt_l[ktp]
                    for s in range(2):
                        kt = ktp * 2 + s
                        for j in range(4):
                            # j0/j1 (bank A) and j2/j3 (bank B) each form one
                            # accumulation group: start only on the bank's
                            # first matmul, stop on its last.
                            nc.tensor.matmul(
                                pvs[:, OFFJ[j]:OFFJ[j] + P + 1],
                                lhsT=pt[:, s * 512 + j * P:s * 512 + (j + 1) * P],
                                rhs=vbig[:, kt * VSTR + g * (P + 1):
                                         kt * VSTR + (g + 1) * (P + 1)],
                                start=(kt == 0 and j % 2 == 0),
                                stop=(kt == NTT - 1 and j % 2 == 1),
                                skip_group_check=True)

                emit_mms(0)
                for ktp in range(8):
                    if ktp < 7:
                        emit_mms(ktp + 1)
                    if ktp < len(fillers) and fillers[ktp] is not None:
                        fillers[ktp]()
                    emit_pv(ktp)

                # emit this head's normalize now so it sits ahead of the next
                # slot's filler casts in the DVE queue: the pv banks then free
                # ~1us into the next slot, before its first PV matmul.
                normalize(qc, h, pvs)

            # ---------------- phase 1: projections + rope -------------------
            with ExitStack() as ph1:
                wq_pool = ph1.enter_context(tc.tile_pool(name="wq", bufs=NKT))
                wvk_pool = ph1.enter_context(tc.tile_pool(name="wvk", bufs=NKT))
                xc_pool = ph1.enter_context(tc.tile_pool(name="xc", bufs=32))
                tab_pool = ph1.enter_context(tc.tile_pool(name="tab", bufs=1))
                tmp_pool = ph1.enter_context(tc.tile_pool(name="rtmp", bufs=2))

                # --- DMA emission: sync + gpsimd queues only, so the scalar
                # (ACT) instruction stream stays free for psum evictions ---
                xc_tiles = {}

                def load_xc(nch, kts, eng):
                    c0 = nch * 512
                    for kt in kts:
                        tl = xc_pool.tile([P, 512], bf16, tag="xc", name=f"xc{nch}_{kt}")
                        eng.dma_start(out=tl[:], in_=xt[kt * P:(kt + 1) * P, c0:c0 + 512])
                        xc_tiles[(nch, kt)] = tl

                # per-core HBM (~358 GB/s) is the startup wall: put ONLY the
                # vproj gate (xc0 on sync, wvk on gpsimd) in the first window,
                # then tables, then wq, then xc1.
                load_xc(0, range(NKT), nc.sync)
                wvk_t = []
                for kt in range(NKT):
                    tl = wvk_pool.tile([P, 2 * GDIM], bf16, tag="wvk", name=f"wvkt{kt}")
                    nc.gpsimd.dma_start(out=tl[:], in_=wvk[kt * P:(kt + 1) * P, :])
                    wvk_t.append(tl)
                wv_t = [tl[:, 0:GDIM] for tl in wvk_t]
                wk_t = [tl[:, GDIM:2 * GDIM] for tl in wvk_t]
                cos_t = tab_pool.tile([P, T], dt.float16, tag="cos", name="cos_t")
                nc.gpsimd.dma_start(out=cos_t[:], in_=cos2[:, :])
                sin_t = tab_pool.tile([P, T], dt.float16, tag="sin", name="sin_t")
                nc.gpsimd.dma_start(out=sin_t[:], in_=sinA[:, :])
                wq_t = [wq_pool.tile([P, QDIM], bf16, tag="wq", name=f"wqt{kt}")
                        for kt in range(NKT)]
                for kt in range(6):
                    nc.sync.dma_start(out=wq_t[kt][:], in_=wq[kt * P:(kt + 1) * P, :])
                for kt in range(6, NKT):
                    nc.gpsimd.dma_start(out=wq_t[kt][:], in_=wq[kt * P:(kt + 1) * P, :])
                load_xc(1, range(NKT), nc.sync)

                def vproj_pair(c, pair):
                    # two t-tiles of 128 into one [128,512] psum; ACT evicts
                    ps = op_tile()
                    for tl_i in (0, 1):
                        for kt in range(NKT):
                            nc.tensor.matmul(
                                ps[:, tl_i * 256:tl_i * 256 + GDIM],
                                lhsT=xc_tiles[(c, kt)][:, (pair * 2 + tl_i) * P:
                                                       (pair * 2 + tl_i + 1) * P],
                                rhs=wv_t[kt],
                                start=(kt == 0), stop=(kt == NKT - 1))
                    for tl_i in (0, 1):
                        tglob = c * 4 + pair * 2 + tl_i
                        for g in range(NG):
                            nc.scalar.activation(
                                vbig[:, tglob * VSTR + g * (P + 1):
                                     tglob * VSTR + g * (P + 1) + P],
                                ps[:, tl_i * 256 + g * P:tl_i * 256 + (g + 1) * P],
                                AF.Copy)

                def rope_evict(m, ps, c0):
                    # qkT[m][:, c0:c0+512] = ps*cos2 + rot(ps)*sinA   (DVE, f32)
                    t1 = tmp_pool.tile([P, 512], f32, tag="t1", name="t1")
                    t2 = tmp_pool.tile([P, 512], f32, tag="t2", name="t2")
                    h2 = P // 2
                    nc.vector.tensor_mul(t1[:], ps[:], cos_t[:, c0:c0 + 512])
                    nc.vector.tensor_mul(
                        t2[0:h2, :], ps[h2:P, :], sin_t[0:h2, c0:c0 + 512])
                    nc.vector.tensor_mul(
                        t2[h2:P, :], ps[0:h2, :], sin_t[h2:P, c0:c0 + 512])
                    nc.vector.tensor_add(qkT[m][:, c0:c0 + 512], t1[:], t2[:])

                def kq_mgroup(c, m):
                    # m 0..7 -> q head m (wq cols), m 8..9 -> k group (wk cols)
                    ps = op_tile()
                    for kt in range(NKT):
                        if m < NH:
                            lhsT = wq_t[kt][:, m * P:(m + 1) * P]
                        else:
                            lhsT = wk_t[kt][:, (m - NH) * P:(m - NH + 1) * P]
                        nc.tensor.matmul(
                            ps[:], lhsT=lhsT,
                            rhs=xc_tiles[(c, kt)][:],
                            start=(kt == 0), stop=(kt == NKT - 1))
                    rope_evict(m, ps, c * 512)

                for c in range(NCH):
                    if 1 <= c < NCH - 1:
                        load_xc(c + 1, range(NKT), nc.sync)
                    vproj_pair(c, 0)
                    vproj_pair(c, 1)
                    # K first (enables attention right after chunk 3's K)
                    for m in (NH, NH + 1):
                        kq_mgroup(c, m)
                    if c < NCH - 1:
                        for m in range(NH):
                            kq_mgroup(c, m)

                # ---- hybrid: chunk-0 attention, Q3 m-groups as PE filler ----
                q3 = {}

                def q3_unit(h, quarter):
                    # quarter of Q-projection m-group h for chunk 3 (4 MMs)
                    c = NCH - 1
                    if quarter == 0:
                        q3[h] = op_tile()
                    ps = q3[h]
                    for kt in range(quarter * 4, quarter * 4 + 4):
                        nc.tensor.matmul(
                            ps[:], lhsT=wq_t[kt][:, h * P:(h + 1) * P],
                            rhs=xc_tiles[(c, kt)][:],
                            start=(kt == 0), stop=(kt == NKT - 1))
                    if quarter == 3:
                        rope_evict(h, ps, c * 512)

                for h in range(NH):
                    fillers = [None] * 8
                    for q in range(4):
                        fillers[1 + 2 * q] = (lambda hh=h, qq=q: q3_unit(hh, qq))
                    attn_slot(0, h, fillers)

            # ---------------- phase 2: chunks 1-3 + o_proj ------------------
            wo_pool = ctx.enter_context(tc.tile_pool(name="wo", bufs=NDT))
            aT_pool = ctx.enter_context(tc.tile_pool(name="aT", bufs=1))
            osb_pool = ctx.enter_context(tc.tile_pool(name="osb", bufs=6))

            # aTbig[:, dtile*T + qt*128 : +128] = attn_t[qt][:, dtile].T
            aTbig = aT_pool.tile([P, NDT * T], bf16, tag="aT")
            wo_t = []
            for dtile in range(NDT):
                tl = wo_pool.tile([P, D_MODEL], bf16, tag="wo", name=f"wot{dtile}")
                wo_t.append(tl)
            for dtile in range(NDT):
                nc.gpsimd.dma_start(out=wo_t[dtile][:, 0:1024],
                                    in_=wo[dtile * P:(dtile + 1) * P, 0:1024])
            for dtile in range(NDT):
                nc.gpsimd.dma_start(out=wo_t[dtile][:, 1024:2048],
                                    in_=wo[dtile * P:(dtile + 1) * P, 1024:2048])

            scr_state = {"tile": None}

            def transp_unit(qcp, tt, pair):
                # transpose attn_t[qt] dtiles (2*pair, 2*pair+1) into aTbig.
                # One native-bf16 psum tile in the op tag (same 2KB slot size)
                # holds all 8 transposes of a t-tile: plain slices keep the
                # subtile dep tracking precise (a bitcast view here serialized
                # every transpose against the previous region's DVE copy).
                # The transpose's start=True bank-clear cannot disturb any
                # in-flight accumulation since the tile owns its bank.
                qt = qcp * 4 + tt
                if pair == 0:
                    scr_state["tile"] = psum.tile(
                        [P, NDT * P], bf16, tag="op", bufs=2, name="scrt")
                scrt = scr_state["tile"]
                for s2 in range(2):
                    dtile = pair * 2 + s2
                    scr = scrt[:, dtile * P:(dtile + 1) * P]
                    nc.tensor.transpose(
                        scr, attn_t[qt][:, dtile * P:(dtile + 1) * P], identity[:])
                    nc.vector.tensor_copy(
                        aTbig[:, dtile * T + qt * P:dtile * T + (qt + 1) * P], scr)

            op_state = {}

            def oproj_unit(tt, nchn, half):
                # half 0: dtiles 0-3 (start); half 1: dtiles 4-7 (stop+evict)
                if half == 0:
                    op_state[(tt, nchn)] = op_tile()
                ps = op_state[(tt, nchn)]
                for dtile in range(half * 4, half * 4 + 4):
                    nc.tensor.matmul(
                        ps[:],
                        lhsT=aTbig[:, dtile * T + tt * P:dtile * T + (tt + 1) * P],
                        rhs=wo_t[dtile][:, nchn * 512:(nchn + 1) * 512],
                        start=(dtile == 0), stop=(dtile == NDT - 1))
                if half == 1:
                    del op_state[(tt, nchn)]
                    osb = osb_pool.tile([P, 512], bf16, tag="osb", name="osb")
                    # alternate evict engine: ACT alone would sit at 10.3us of
                    # an 11.2us slot (8 exps + 2 casts), DVE has ~6us slack
                    if (tt + nchn) % 2 == 0:
                        nc.scalar.activation(osb[:], ps[:], AF.Copy)
                    else:
                        nc.vector.tensor_copy(osb[:], ps[:])
                    # alternate output queues (sync is idle during phase 2)
                    eng = nc.gpsimd if (tt + nchn) % 2 == 0 else nc.sync
                    eng.dma_start(
                        out=out[tt * P:(tt + 1) * P, nchn * 512:(nchn + 1) * 512],
                        in_=osb[:])

            # filler scheduling: per chunk qc (1..3), slots h=0..7 carry
            # transposes of chunk qc-1 (slot h<4 -> tt=h, 4 pair-units at
            # points 4-7) and o_proj groups of chunk qc-1 (2 units each) from
            # a readiness queue.
            ready_groups = []   # (tt_glob, nchn) ready once tt transposed

            for qc in range(1, NCH):
                qcp = qc - 1
                for h in range(NH):
                    fillers = []
                    trans = []
                    if h < 4:
                        trans = [(lambda t=h, p=p2: transp_unit(qcp, t, p))
                                 for p2 in range(4)]
                    n_op = min(8 - len(trans), 4)
                    opu = []
                    while ready_groups and len(opu) + 2 <= n_op:
                        ttg, nchn = ready_groups.pop(0)
                        opu.append(lambda a=ttg, b=nchn: oproj_unit(a, b, 0))
                        opu.append(lambda a=ttg, b=nchn: oproj_unit(a, b, 1))
                    # op units first (points 0..), transposes at the tail
                    fillers = opu + trans
                    attn_slot(qc, h, fillers)
                    if h < 4:
                        ttg = qcp * 4 + h
                        for nchn in range(NCH):
                            ready_groups.append((ttg, nchn))

            # ---------------- tail: transposes + o_proj of the last chunk ---
            # transposes pace at ~400ns each (LDWEIGHTS + SBUF latency), so
            # interleave every transpose unit with a ready o_proj half-group;
            # the held-back leftover groups prime the queue while the last
            # head's exp/PV/normalize chain drains.
            qcp = NCH - 1
            units = []          # pending o_proj half-group emissions

            def pump():
                if units:
                    units.pop(0)()

            for ttg, nchn in ready_groups:
                for hf in (0, 1):
                    units.append(lambda a=ttg, b=nchn, c=hf: oproj_unit(a, b, c))
            ready_groups = []
            for tt in range(4):
                for p2 in range(4):
                    transp_unit(qcp, tt, p2)
                    pump()
                for nchn in range(NCH):
                    for hf in (0, 1):
                        units.append(lambda a=qcp * 4 + tt, b=nchn, c=hf:
                                     oproj_unit(a, b, c))
            while units:
                pump()

    nc.compile()
    return nc


def make_tables():
    inv_freq = 1.0 / (THETA ** (np.arange(0, HEAD_DIM, 2, dtype=np.float32)
                                / HEAD_DIM))          # [64]
    ang = np.arange(T, dtype=np.float32)[:, None] * inv_freq[None, :]  # [T, 64]
    cos = np.cos(ang).T.astype(np.float32)            # [64, T]
    sin = np.sin(ang).T.astype(np.float32)
    cos2 = np.concatenate([cos, cos], axis=0)         # [128, T]
    sinA = np.concatenate([-sin, sin], axis=0)        # [128, T]
    return (np.ascontiguousarray(cos2).astype(np.float16),
            np.ascontiguousarray(sinA).astype(np.float16))


def make_in_maps(x, W_qkv, W_o, padding_mask, masked):
    cos2_v, sinA_v = make_tables()
    in_maps = []
    for c in range(N_CORES):
        b, half = c // 2, c % 2
        q0 = half * QDIM
        k0 = NUM_HEADS * HEAD_DIM + half * GDIM
        v0 = NUM_HEADS * HEAD_DIM + QUERY_GROUPS * HEAD_DIM + half * GDIM
        wvk_v = np.concatenate(
            [W_qkv[:, v0:v0 + GDIM], W_qkv[:, k0:k0 + GDIM]], axis=1)
        m = {
            "xt": np.ascontiguousarray(x[b].T).astype(BF16),
            "wq": np.ascontiguousarray(W_qkv[:, q0:q0 + QDIM]).astype(BF16),
            "wvk": np.ascontiguousarray(wvk_v).astype(BF16),
            "wo": np.ascontiguousarray(W_o[half * QDIM:(half + 1) * QDIM, :]).astype(BF16),
            "cos2": cos2_v, "sinA": sinA_v,
        }
        if masked:
            bias = np.where(padding_mask[b], 0.0, -1e30).astype(np.float32)  # [T]
            m["maskcol"] = np.ascontiguousarray(
                bias.reshape(NTT, P).T).astype(np.float32)
        in_maps.append(m)
    return in_maps


_nc_cache = {}


def kernel(x, W_qkv, W_o, padding_mask, trace=False):
    from concourse.bass_utils import run_bass_kernel_spmd

    x = np.asarray(x)
    W_qkv = np.asarray(W_qkv)
    W_o = np.asarray(W_o)
    padding_mask = np.asarray(padding_mask)
    masked = not bool(padding_mask.all())

    if masked not in _nc_cache:
        _nc_cache[masked] = build_nc(masked)
    nc = _nc_cache[masked]

    in_maps = make_in_maps(x, W_qkv, W_o, padding_mask, masked)
    res = run_bass_kernel_spmd(
        nc, in_maps, core_ids=list(range(N_CORES)),
        trace=trace, trace_cores=[0] if trace else None)

    out = np.empty((B, T, D_MODEL), np.float32)
    for b in range(B):
        out[b] = (res.results[2 * b]["out"].astype(np.float32)
                  + res.results[2 * b + 1]["out"].astype(np.float32))
    kernel.last_exec_time_ns = res.exec_time_ns
    kernel.last_results = res
    return out
